# revision 14
# baseline (speedup 1.0000x reference)
"""GPT-2 block (B=2, T=2048, C=768, H=12) on 8 Trainium2 NeuronCores.

Sharding: data-parallel over batch (2) x 4-way query-tile split per batch.
Each core computes K/V for its full batch (avoids on-chip collectives,
whose latency floor exceeds the redundant compute) and runs attention +
MLP for 4 of the 16 query tiles, interleaved {g, 7-g, 8+g, 15-g} so the
causal-attention work is identical across cores.

The SPMD program is uniform across cores: per-core differences are pushed
into the data via a k-tile permutation of the sequence (each core's query
tiles sit at fixed positions {3,7,11,15}; every tile's causal prefix is
placed before it) plus per-core causal masks.

Layouts: activations enter matmuls feature-major (xnT [C,T]) so QKV needs
no transposes; attention scores are computed transposed (S^T [k,q]) so
exp(S^T) is directly the stationary operand of the A*V matmul, and a ones
column appended to V produces the softmax denominator in the same matmul.
"""

import sys

sys.path.insert(0, "/opt/trn_rl_repo")

import numpy as np
import ml_dtypes

import bass_rust
import concourse.bass as bass
import concourse.bacc as bacc
import concourse.tile as tile
from concourse import mybir
from concourse.vector_clock import ScopedClock

BF16 = ml_dtypes.bfloat16
F32 = mybir.dt.float32
BF = mybir.dt.bfloat16

B, T, C, H = 2, 2048, 768, 12
HD = C // H  # 64
DFF = 4 * C  # 3072
TT = T // 128  # 16 token tiles
CT = C // 128  # 6 feature tiles
FT = DFF // 128  # 24
QPOS = (3, 7, 11, 15)  # fixed positions of this core's query tiles
NQ = 512  # queries per core
AF = mybir.ActivationFunctionType
ALU = mybir.AluOpType

# ---------------------------------------------------------------------------
# Tile exit-drain fix: the final SP drain carries one wait per live logical
# processor, but TRN2 ISA instructions hold at most 1 embedded sync wait in
# this toolchain. Split the waits across a chain of SP drains.
# ---------------------------------------------------------------------------
_MAX_WAITS = 1


def _drain_and_barrier(self, tick_clock, wait_clock):
    drain_inst = self.nc.sync.drain()
    wait_clock.add_sem_waits(
        drain_inst.ins, ScopedClock({None: tick_clock.global_clock})
    )
    si = drain_inst.ins.sync_info
    if si is not None and len(si.on_wait) > _MAX_WAITS:
        waits = list(si.on_wait)
        drain_inst.ins.sync_info = bass_rust.SyncInfo(
            on_wait=waits[:_MAX_WAITS], on_update=list(si.on_update)
        )
        rest = waits[_MAX_WAITS:]
        for i in range(0, len(rest), _MAX_WAITS):
            extra = self.nc.sync.drain()
            extra.ins.sync_info = bass_rust.SyncInfo(
                on_wait=rest[i : i + _MAX_WAITS], on_update=[]
            )
    self.nc.all_engine_barrier()
    assert self.sems is not None
    popped = self.nc._tile_sem_poison_stack.pop()
    assert popped is self._sem_poison
    self.nc.clear_and_free_semaphores(list(self.sems.allocated().values()))
    self.nc.all_engine_barrier()


tile.TileContext._drain_and_barrier = _drain_and_barrier


# ---------------------------------------------------------------------------
# Per-core sharding layout (host side)
# ---------------------------------------------------------------------------
def core_layout(g):
    """For group index g (0..3): (qtiles sorted, perm) with the core's query
    tiles at positions QPOS and every tile's causal prefix placed before it."""
    qtiles = sorted([g, 7 - g, 8 + g, 15 - g])
    posmap = dict(zip(QPOS, qtiles))
    rest = iter([t for t in range(TT) if t not in qtiles])
    perm = [posmap[p] if p in posmap else next(rest) for p in range(TT)]
    # causal validity: tiles <= qtiles[j] all sit at positions <= QPOS[j]
    for j, a in enumerate(qtiles):
        assert set(range(a + 1)) <= set(perm[: QPOS[j] + 1]), (g, j, perm)
    return qtiles, perm


def core_masks(qtiles, perm):
    """masks[kp] = causal mask of k-position kp against query tile j=kp//4
    (the first in-suffix block — across all core layouts the only block
    that is ever not all-ones)."""
    masks = np.zeros((TT, 128, 128), dtype=BF16)
    for kp in range(TT):
        tk = perm[kp] * 128 + np.arange(128)[:, None]
        a = qtiles[kp // 4]
        tq = a * 128 + np.arange(128)[None, :]
        masks[kp] = (tk <= tq).astype(BF16)
    return masks


# ---------------------------------------------------------------------------
# The Bass program (identical for all 8 cores)
# ---------------------------------------------------------------------------
def build_program():
    nc = bacc.Bacc("TRN2")

    d_x = nc.dram_tensor("x_perm", [T, C], F32, kind="ExternalInput")
    d_xob = nc.dram_tensor("x_own_b", [NQ, C], F32, kind="ExternalInput")
    d_masks = nc.dram_tensor("masks", [TT, 128, 128], BF, kind="ExternalInput")
    d_wq = nc.dram_tensor("wq", [C, C], BF, kind="ExternalInput")
    d_wk = nc.dram_tensor("wk", [C, C], BF, kind="ExternalInput")
    d_wv = nc.dram_tensor("wv", [C, C], BF, kind="ExternalInput")
    d_wp = nc.dram_tensor("wp", [C, C], BF, kind="ExternalInput")
    d_wfc = nc.dram_tensor("wfc", [C, DFF], BF, kind="ExternalInput")
    d_wfc2 = nc.dram_tensor("wfc2", [DFF, C], BF, kind="ExternalInput")
    d_bq = nc.dram_tensor("bq2d", [128, CT], F32, kind="ExternalInput")
    d_bk = nc.dram_tensor("bk2d", [128, CT], F32, kind="ExternalInput")
    d_bv = nc.dram_tensor("bv_bc", [128, C], F32, kind="ExternalInput")
    d_bfc = nc.dram_tensor("bfc_bc", [128, DFF], F32, kind="ExternalInput")
    d_bfc2 = nc.dram_tensor("bfc2_bc", [128, C], F32, kind="ExternalInput")
    d_l1g = nc.dram_tensor("ln1g2d", [128, CT], F32, kind="ExternalInput")
    d_l1b = nc.dram_tensor("ln1b2d", [128, CT], F32, kind="ExternalInput")
    d_l2g = nc.dram_tensor("ln2g2d", [128, CT], F32, kind="ExternalInput")
    d_l2b = nc.dram_tensor("ln2b2d", [128, CT], F32, kind="ExternalInput")
    d_ident = nc.dram_tensor("ident", [128, 128], BF, kind="ExternalInput")
    d_out = nc.dram_tensor("out", [NQ, C], F32, kind="ExternalOutput")

    with tile.TileContext(nc) as tc:
        _body(nc, tc, locals())
    nc.compile()
    return nc


def _ln_tile(nc, pool, x_ap, eps):
    """LayerNorm stats for one [128, C] fp32 tile -> (mu, rstd) [128,1]."""
    stats = pool.tile([128, 3, 6], F32, tag="bnstats", name="bnstats")
    xg = x_ap.rearrange("p (a b) -> p a b", b=256)
    for a in range(3):
        nc.vector.bn_stats(out=stats[:, a, :], in_=xg[:, a, :])
    mv = pool.tile([128, 2], F32, tag="bnaggr", name="bnaggr")
    nc.vector.bn_aggr(out=mv[:], in_=stats[:])
    sd = pool.tile([128, 1], F32, tag="sd", name="sd")
    nc.scalar.activation(out=sd[:], in_=mv[:, 1:2], func=AF.Sqrt, bias=eps[:])
    rstd = pool.tile([128, 1], F32, tag="rstd", name="rstd")
    nc.vector.reciprocal(out=rstd[:], in_=sd[:])
    return mv[:, 0:1], rstd


def _body(nc, tc, d):
    ctx_pools = []

    def pool(name, **kw):
        return tc.tile_pool(name=name, **kw)

    with (
        pool("const", bufs=1) as constp,
        pool("persist", bufs=1) as pers,
        pool("small", bufs=4) as small,
    ):
        # ---- constants -------------------------------------------------
        ident = constp.tile([128, 128], BF)
        nc.sync.dma_start(ident[:], d["d_ident"][:])
        eps = constp.tile([128, 1], F32)
        nc.vector.memset(eps[:], 1e-5)
        l1g = constp.tile([128, CT], F32)
        nc.sync.dma_start(l1g[:], d["d_l1g"][:])
        l1b = constp.tile([128, CT], F32)
        nc.sync.dma_start(l1b[:], d["d_l1b"][:])
        l2g = constp.tile([128, CT], F32)
        nc.sync.dma_start(l2g[:], d["d_l2g"][:])
        l2b = constp.tile([128, CT], F32)
        nc.sync.dma_start(l2b[:], d["d_l2b"][:])
        bq = constp.tile([128, CT], F32)
        nc.sync.dma_start(bq[:], d["d_bq"][:])
        bk = constp.tile([128, CT], F32)
        nc.sync.dma_start(bk[:], d["d_bk"][:])
        bv_bc = constp.tile([128, C], F32)
        nc.sync.dma_start(bv_bc[:], d["d_bv"][:])
        bfc2_bc = constp.tile([128, C], F32)
        nc.sync.dma_start(bfc2_bc[:], d["d_bfc2"][:])
        ones64 = constp.tile([1, 64], F32)
        nc.vector.memset(ones64[:], 1.0)

        # ---- persistent activations -----------------------------------
        x_own = [pers.tile([128, C], F32, tag=f"xo{j}", name=f"xo{j}") for j in range(4)]
        for j in range(4):
            nc.sync.dma_start(x_own[j][:], d["d_xob"][j * 128 : (j + 1) * 128, :])
        qT = [pers.tile([128, NQ], BF, tag=f"qT{t}", name=f"qT{t}") for t in range(CT)]
        yT = [pers.tile([128, NQ], BF, tag=f"yT{t}", name=f"yT{t}") for t in range(CT)]

        with pool("attn_sb", bufs=1) as attnp:
            kT = [attnp.tile([128, T], BF, tag=f"kT{t}", name=f"kT{t}") for t in range(CT)]
            V = [attnp.tile([128, H * (HD + 1)], BF, tag=f"V{t}", name=f"V{t}") for t in range(TT)]
            masks = [attnp.tile([128, 128], BF, tag=f"m{t}", name=f"m{t}") for t in range(TT)]
            for t in range(TT):
                nc.sync.dma_start(masks[t][:], d["d_masks"][t, :, :])

            # ======== phase 1: LN1 + transpose to feature-major ========
            with (
                pool("ph1", bufs=1) as ph1p,
                pool("ph1s", bufs=3) as ph1s,
            ):
                xnT = [ph1p.tile([128, T], BF, tag=f"xnT{t}", name=f"xnT{t}") for t in range(CT)]
                xnTq = [ph1p.tile([128, NQ], BF, tag=f"xnTq{t}", name=f"xnTq{t}") for t in range(CT)]
                with pool("ph1t", bufs=1, space="PSUM") as ph1t:
                    for ttg in range(4):
                        ptb = [
                            ph1t.tile([128, 512], BF, tag=f"ptb{t}", name=f"ptb{t}")
                            for t in range(CT)
                        ]
                        for ti in range(4):
                            tt = ttg * 4 + ti
                            xt = ph1s.tile([128, C], F32, tag="xt", name="xt")
                            nc.sync.dma_start(xt[:], d["d_x"][tt * 128 : (tt + 1) * 128, :])
                            mu, rstd = _ln_tile(nc, small, xt[:], eps)
                            xn = ph1s.tile([128, C], BF, tag="xn", name="xn")
                            nc.vector.tensor_scalar(
                                out=xn[:], in0=xt[:], scalar1=mu, scalar2=rstd[:],
                                op0=ALU.subtract, op1=ALU.mult,
                            )
                            for ct in range(CT):
                                nc.tensor.transpose(
                                    ptb[ct][:, ti * 128 : (ti + 1) * 128],
                                    xn[:, ct * 128 : (ct + 1) * 128], ident[:],
                                )
                        for ct in range(CT):
                            nc.scalar.activation(
                                out=xnT[ct][:, ttg * 512 : (ttg + 1) * 512],
                                in_=ptb[ct][:], func=AF.Identity,
                                scale=l1g[:, ct : ct + 1], bias=l1b[:, ct : ct + 1],
                            )
                            # own q-tile of this group sits at ti == 3
                            nc.scalar.activation(
                                out=xnTq[ct][:, ttg * 128 : (ttg + 1) * 128],
                                in_=ptb[ct][:, 384:512], func=AF.Identity,
                                scale=l1g[:, ct : ct + 1], bias=l1b[:, ct : ct + 1],
                            )

                # ======== phase 2: Q^T, K^T, V ========
                with (
                    pool("wqkv", bufs=1) as wp_,
                    pool("ph2ps", bufs=2, space="PSUM") as ph2ps,
                    pool("ph2pv", bufs=2, space="PSUM") as ph2pv,
                ):
                    wq = [wp_.tile([128, C], BF, tag=f"wq{t}", name=f"wq{t}") for t in range(CT)]
                    wk = [wp_.tile([128, C], BF, tag=f"wk{t}", name=f"wk{t}") for t in range(CT)]
                    wv = [wp_.tile([128, C], BF, tag=f"wv{t}", name=f"wv{t}") for t in range(CT)]
                    for t in range(CT):
                        sl = slice(t * 128, (t + 1) * 128)
                        nc.sync.dma_start(wq[t][:], d["d_wq"][sl, :])
                        nc.sync.dma_start(wk[t][:], d["d_wk"][sl, :])
                        nc.sync.dma_start(wv[t][:], d["d_wv"][sl, :])

                    # kT chunk nn and V tiles 4nn..4nn+3 depend only on
                    # token-group nn of xnT -> overlap with phase 1 tail
                    for nn in range(4):
                        for f in range(CT):
                            ps = ph2ps.tile([128, 512], F32, tag="pqk", name="pk")
                            for ct in range(CT):
                                nc.tensor.matmul(
                                    ps[:], wk[ct][:, f * 128 : (f + 1) * 128],
                                    xnT[ct][:, nn * 512 : (nn + 1) * 512],
                                    start=(ct == 0), stop=(ct == CT - 1),
                                )
                            nc.vector.tensor_scalar(
                                out=kT[f][:, nn * 512 : (nn + 1) * 512],
                                in0=ps[:], scalar1=bk[:, f : f + 1],
                                scalar2=None, op0=ALU.add,
                            )
                        for tt in range(nn * 4, nn * 4 + 4):
                            pv = ph2pv.tile([128, C], F32, tag="pv", name="pv")
                            for lo, hi in ((0, 512), (512, 768)):
                                for ct in range(CT):
                                    nc.tensor.matmul(
                                        pv[:, lo:hi],
                                        xnT[ct][:, tt * 128 : (tt + 1) * 128],
                                        wv[ct][:, lo:hi],
                                        start=(ct == 0), stop=(ct == CT - 1),
                                    )
                            vt = V[tt][:].rearrange("p (h e) -> p h e", e=HD + 1)
                            nc.vector.memset(vt[:, :, HD : HD + 1], 1.0)
                            pvh = pv[:].rearrange("p (h e) -> p h e", e=HD)
                            nc.vector.tensor_tensor(
                                out=vt[:, :, 0:HD], in0=pvh[:],
                                in1=bv_bc[:].rearrange("p (h e) -> p h e", e=HD),
                                op=ALU.add,
                            )
                    # qT[f] [128, 512] = (Wq[:, f].T @ xnTq) + bq
                    for f in range(CT):
                        ps = ph2ps.tile([128, NQ], F32, tag="pqk", name="pq")
                        for ct in range(CT):
                            nc.tensor.matmul(
                                ps[:], wq[ct][:, f * 128 : (f + 1) * 128],
                                xnTq[ct][:], start=(ct == 0), stop=(ct == CT - 1),
                            )
                        nc.vector.tensor_scalar(
                            out=qT[f][:], in0=ps[:], scalar1=bq[:, f : f + 1],
                            scalar2=None, op0=ALU.add,
                        )

            # ======== phase 3: attention ========
            with (
                pool("ph3", bufs=8) as ph3s,
                pool("ph3ps", bufs=3, space="PSUM") as ph3ps,
                pool("ph3pa", bufs=1, space="PSUM") as ph3pa,
            ):
                # 4 heads in flight per kp step: PE's in-order stream gets
                # 3 other S-matmuls between a head's S and its AV, hiding the
                # exp(ACT) + mask(DVE) latency.
                for hg in range(H // 4):
                    hs = [hg * 4 + i for i in range(4)]
                    pavs = {
                        h: ph3pa.tile(
                            [128, NQ], F32, tag=f"pav{h % 4}", name=f"pav{h % 4}"
                        )
                        for h in hs
                    }
                    for kp in range(TT):
                        cs = 128 * (kp // 4)
                        pss, psbs = {}, {}
                        for h in hs:
                            ro = (h % 2) * 64
                            ps = ph3ps.tile([128, NQ], F32, tag="ps", name="ps")
                            nc.tensor.matmul(
                                ps[:, cs:NQ],
                                kT[h // 2][ro : ro + 64, kp * 128 : (kp + 1) * 128],
                                qT[h // 2][ro : ro + 64, cs:NQ],
                            )
                            pss[h] = ps
                            p_sb = ph3s.tile([128, NQ], BF, tag="p_sb", name="p_sb")
                            nc.scalar.activation(
                                out=p_sb[:, cs:NQ], in_=ps[:, cs:NQ],
                                func=AF.Exp, scale=0.125,
                            )
                            # only the first in-suffix 128-col block is ever
                            # not all-ones (across every core layout)
                            nc.vector.tensor_mul(
                                p_sb[:, cs : cs + 128], p_sb[:, cs : cs + 128],
                                masks[kp][:],
                            )
                            psbs[h] = p_sb
                        for h in hs:
                            nc.tensor.matmul(
                                pavs[h][0 : HD + 1, cs:NQ],
                                V[kp][:, h * (HD + 1) : (h + 1) * (HD + 1)],
                                psbs[h][:, cs:NQ],
                                start=(kp == 0), stop=(kp == TT - 1),
                                skip_group_check=True,
                            )
                    for h in hs:
                        ro = (h % 2) * 64
                        pav = pavs[h]
                        den = ph3s.tile([1, NQ], F32, tag="den", name="den")
                        nc.scalar.copy(den[:], pav[HD : HD + 1, :])
                        rbp = ph3ps.tile([64, NQ], F32, tag="rbp", name="rbp", bufs=1)
                        nc.tensor.matmul(rbp[:], ones64[:], den[:])
                        rb = ph3s.tile([64, NQ], F32, tag="rb", name="rb")
                        nc.vector.reciprocal(out=rb[:], in_=rbp[:])
                        nc.vector.tensor_tensor(
                            out=yT[h // 2][ro : ro + 64, :], in0=pav[0:HD, :],
                            in1=rb[:], op=ALU.mult,
                        )

        # ======== phase 4: proj + residual + LN2 ========
        with pool("mlp_sb", bufs=1) as mlpp:
            x2 = [mlpp.tile([128, C], F32, tag=f"x2{j}", name=f"x2{j}") for j in range(4)]
            xn2T = [mlpp.tile([128, NQ], BF, tag=f"xn2T{t}", name=f"xn2T{t}") for t in range(CT)]
            hT = [mlpp.tile([128, NQ], BF, tag=f"hT{t}", name=f"hT{t}") for t in range(FT)]
            with (
                pool("mlp1", bufs=1) as m1p,
                pool("mlp1s", bufs=3) as m1s,
            ):
                wp = [m1p.tile([128, C], BF, tag=f"wp{t}", name=f"wp{t}") for t in range(CT)]
                wfc = [m1p.tile([128, DFF], BF, tag=f"wfc{t}", name=f"wfc{t}") for t in range(CT)]
                bfc_bc = m1p.tile([128, DFF], F32, tag="bfcbc", name="bfcbc")
                nc.sync.dma_start(bfc_bc[:], d["d_bfc"][:])
                hh = [m1p.tile([128, DFF], BF, tag=f"hh{j}", name=f"hh{j}") for j in range(4)]
                for t in range(CT):
                    sl = slice(t * 128, (t + 1) * 128)
                    nc.sync.dma_start(wp[t][:], d["d_wp"][sl, :])
                    nc.sync.dma_start(wfc[t][:], d["d_wfc"][sl, :])

                with (
                    pool("ph4p", bufs=2, space="PSUM") as ph4p,
                    pool("ph4t", bufs=4, space="PSUM") as ph4t,
                ):
                    for qt in range(4):
                        pp = ph4p.tile([128, C], F32, tag="pp", name="pp")
                        for lo, hi in ((0, 512), (512, 768)):
                            for ct in range(CT):
                                nc.tensor.matmul(
                                    pp[:, lo:hi],
                                    yT[ct][:, qt * 128 : (qt + 1) * 128],
                                    wp[ct][:, lo:hi],
                                    start=(ct == 0), stop=(ct == CT - 1),
                                )
                        nc.vector.tensor_add(x2[qt][:], pp[:], x_own[qt][:])
                        mu, rstd = _ln_tile(nc, small, x2[qt][:], eps)
                        xn2 = m1s.tile([128, C], BF, tag="xn2", name="xn2")
                        nc.vector.tensor_scalar(
                            out=xn2[:], in0=x2[qt][:], scalar1=mu, scalar2=rstd[:],
                            op0=ALU.subtract, op1=ALU.mult,
                        )
                        for ct in range(CT):
                            pt = ph4t.tile([128, 128], BF, tag="pt4", name="pt4")
                            nc.tensor.transpose(
                                pt[:], xn2[:, ct * 128 : (ct + 1) * 128], ident[:]
                            )
                            nc.scalar.activation(
                                out=xn2T[ct][:, qt * 128 : (qt + 1) * 128],
                                in_=pt[:], func=AF.Identity,
                                scale=l2g[:, ct : ct + 1], bias=l2b[:, ct : ct + 1],
                            )

                # ======== phase 5: fc + gelu ========
                with pool("ph5p", bufs=2, space="PSUM") as ph5p:
                    for qt in range(4):
                        for nn in range(6):
                            sl = slice(nn * 512, (nn + 1) * 512)
                            ph_ = ph5p.tile([128, 512], F32, tag="ph5", name="ph5")
                            for ct in range(CT):
                                nc.tensor.matmul(
                                    ph_[:],
                                    xn2T[ct][:, qt * 128 : (qt + 1) * 128],
                                    wfc[ct][:, sl],
                                    start=(ct == 0), stop=(ct == CT - 1),
                                )
                            tmp = m1s.tile([128, 512], BF, tag="pregelu", name="pregelu")
                            nc.vector.tensor_add(tmp[:], ph_[:], bfc_bc[:, sl])
                            nc.scalar.activation(
                                out=hh[qt][:, sl], in_=tmp[:], func=AF.Gelu_apprx_tanh
                            )

                # ======== phase 6: transpose h ========
                with pool("ph6t", bufs=2, space="PSUM") as ph6t:
                    for fc in range(FT):
                        ptb = ph6t.tile([128, 512], BF, tag="pt6", name="pt6")
                        for qt in range(4):
                            nc.tensor.transpose(
                                ptb[:, qt * 128 : (qt + 1) * 128],
                                hh[qt][:, fc * 128 : (fc + 1) * 128], ident[:],
                            )
                        nc.scalar.copy(hT[fc][:], ptb[:])

            # ======== phase 7: fc2 + residual + out ========
            with (
                pool("mlp2", bufs=1) as m2p,
                pool("mlp2s", bufs=3) as m2s,
                pool("ph7p", bufs=2, space="PSUM") as ph7p,
            ):
                wfc2 = [m2p.tile([128, C], BF, tag=f"wfc2{t}", name=f"wfc2{t}") for t in range(FT)]
                for t in range(FT):
                    nc.sync.dma_start(
                        wfc2[t][:], d["d_wfc2"][t * 128 : (t + 1) * 128, :]
                    )
                for qt in range(4):
                    po = ph7p.tile([128, C], F32, tag="po", name="po")
                    for lo, hi in ((0, 512), (512, 768)):
                        for kt in range(FT):
                            nc.tensor.matmul(
                                po[:, lo:hi],
                                hT[kt][:, qt * 128 : (qt + 1) * 128],
                                wfc2[kt][:, lo:hi],
                                start=(kt == 0), stop=(kt == FT - 1),
                            )
                    t1 = m2s.tile([128, C], F32, tag="t1", name="t1")
                    nc.vector.tensor_add(t1[:], po[:], bfc2_bc[:])
                    ot = m2s.tile([128, C], F32, tag="ot", name="ot")
                    nc.vector.tensor_add(ot[:], t1[:], x2[qt][:])
                    nc.sync.dma_start(
                        d["d_out"][qt * 128 : (qt + 1) * 128, :], ot[:]
                    )


# ---------------------------------------------------------------------------
# Host-side wrapper
# ---------------------------------------------------------------------------
_PROGRAM = None


def _get_program():
    global _PROGRAM
    if _PROGRAM is None:
        _PROGRAM = build_program()
    return _PROGRAM


def make_in_maps(x, ln1_g, ln1_b, W_attn, b_attn, W_proj, b_proj,
                 ln2_g, ln2_b, W_fc, b_fc, W_fc2, b_fc2):
    x = np.asarray(x, np.float32)
    shared = {
        "wq": np.asarray(W_attn[:, 0:C], BF16),
        "wk": np.asarray(W_attn[:, C : 2 * C], BF16),
        "wv": np.asarray(W_attn[:, 2 * C : 3 * C], BF16),
        "wp": np.asarray(W_proj, BF16),
        "wfc": np.asarray(W_fc, BF16),
        "wfc2": np.asarray(W_fc2, BF16),
        "bq2d": np.ascontiguousarray(
            np.asarray(b_attn[0:C], np.float32).reshape(CT, 128).T),
        "bk2d": np.ascontiguousarray(
            np.asarray(b_attn[C : 2 * C], np.float32).reshape(CT, 128).T),
        "bv_bc": np.broadcast_to(
            np.asarray(b_attn[2 * C : 3 * C], np.float32), (128, C)).copy(),
        "bfc_bc": np.broadcast_to(
            np.asarray(b_fc, np.float32), (128, DFF)).copy(),
        "bfc2_bc": np.broadcast_to(
            np.asarray(b_fc2, np.float32), (128, C)).copy(),
        "ln1g2d": np.ascontiguousarray(
            np.asarray(ln1_g, np.float32).reshape(CT, 128).T),
        "ln1b2d": np.ascontiguousarray(
            np.asarray(ln1_b, np.float32).reshape(CT, 128).T),
        "ln2g2d": np.ascontiguousarray(
            np.asarray(ln2_g, np.float32).reshape(CT, 128).T),
        "ln2b2d": np.ascontiguousarray(
            np.asarray(ln2_b, np.float32).reshape(CT, 128).T),
        "ident": np.eye(128, dtype=BF16),
    }
    bp = np.asarray(b_proj, np.float32)
    in_maps, layouts = [], []
    for core in range(8):
        b, g = core // 4, core % 4
        qtiles, perm = core_layout(g)
        idx = np.concatenate([np.arange(t * 128, (t + 1) * 128) for t in perm])
        own = np.concatenate([np.arange(t * 128, (t + 1) * 128) for t in qtiles])
        m = dict(shared)
        m["x_perm"] = np.ascontiguousarray(x[b][idx])
        m["x_own_b"] = np.ascontiguousarray(x[b][own] + bp)
        m["masks"] = core_masks(qtiles, perm)
        in_maps.append(m)
        layouts.append((b, own))
    return in_maps, layouts


def unshard(results, layouts):
    out = np.empty((B, T, C), np.float32)
    for r, (b, own) in zip(results, layouts):
        out[b][own] = r["out"]
    return out


def kernel(**inputs):
    from concourse.bass_utils import run_bass_kernel_spmd

    nc = _get_program()
    in_maps, layouts = make_in_maps(**inputs)
    res = run_bass_kernel_spmd(nc, in_maps, core_ids=list(range(8)))
    return unshard(res.results, layouts)


# revision 16
# speedup vs baseline: 1.1471x; 1.1471x over previous
"""GPT-2 block (B=2, T=2048, C=768, H=12) on 8 Trainium2 NeuronCores.

Sharding: data-parallel over batch (2) x 4-way query-tile split per batch.
Each core computes K/V for its full batch (avoids on-chip collectives,
whose latency floor exceeds the redundant compute) and runs attention +
MLP for 4 of the 16 query tiles, interleaved {g, 7-g, 8+g, 15-g} so the
causal-attention work is identical across cores.

The SPMD program is uniform across cores: per-core differences are pushed
into the data via a k-tile permutation of the sequence (each core's query
tiles sit at fixed positions {3,7,11,15}; every tile's causal prefix is
placed before it) plus per-core causal masks.

Layouts: activations enter matmuls feature-major (xnT [C,T]) so QKV needs
no transposes; attention scores are computed transposed (S^T [k,q]) so
exp(S^T) is directly the stationary operand of the A*V matmul, and a ones
column appended to V produces the softmax denominator in the same matmul.

Precision: weights and matmul activations are fp8 e4m3 with DoubleRow
matmuls (2 K-rows/cycle, K-tiles of 256). Weights are pre-scaled by 32 on
the host to stay in the fp8 normal range; the scale folds into the exp()
argument for attention (q,k both 32x -> scale/1024), into the V ones
column (=32 so softmax numerator/denominator cancel), and into one cheap
descale per MLP/proj output. Attention S/AV matmuls stay bf16.
"""

import sys

sys.path.insert(0, "/opt/trn_rl_repo")

import numpy as np
import ml_dtypes

import bass_rust
import concourse.bass as bass
import concourse.bacc as bacc
import concourse.tile as tile
from concourse import mybir
from concourse.vector_clock import ScopedClock

BF16 = ml_dtypes.bfloat16
F32 = mybir.dt.float32
BF = mybir.dt.bfloat16
F8 = mybir.dt.float8e4
NP_F8 = mybir.dt.np(F8)

B, T, C, H = 2, 2048, 768, 12
HD = C // H  # 64
DFF = 4 * C  # 3072
TT = T // 128  # 16 token tiles
CT = C // 128  # 6 feature tiles
KT = C // 256  # 3 DoubleRow k-tiles over C
KT2 = DFF // 256  # 12 DoubleRow k-tiles over DFF
FT = DFF // 128  # 24
QPOS = (3, 7, 11, 15)  # fixed positions of this core's query tiles
NQ = 512  # queries per core
WS = 32.0  # fp8 weight pre-scale
AF = mybir.ActivationFunctionType
ALU = mybir.AluOpType
DR = mybir.MatmulPerfMode.DoubleRow

# ---------------------------------------------------------------------------
# Tile exit-drain fix: the final SP drain carries one wait per live logical
# processor, but TRN2 ISA instructions hold at most 1 embedded sync wait in
# this toolchain. Split the waits across a chain of SP drains.
# ---------------------------------------------------------------------------
_MAX_WAITS = 1


def _drain_and_barrier(self, tick_clock, wait_clock):
    drain_inst = self.nc.sync.drain()
    wait_clock.add_sem_waits(
        drain_inst.ins, ScopedClock({None: tick_clock.global_clock})
    )
    si = drain_inst.ins.sync_info
    if si is not None and len(si.on_wait) > _MAX_WAITS:
        waits = list(si.on_wait)
        drain_inst.ins.sync_info = bass_rust.SyncInfo(
            on_wait=waits[:_MAX_WAITS], on_update=list(si.on_update)
        )
        rest = waits[_MAX_WAITS:]
        for i in range(0, len(rest), _MAX_WAITS):
            extra = self.nc.sync.drain()
            extra.ins.sync_info = bass_rust.SyncInfo(
                on_wait=rest[i : i + _MAX_WAITS], on_update=[]
            )
    self.nc.all_engine_barrier()
    assert self.sems is not None
    popped = self.nc._tile_sem_poison_stack.pop()
    assert popped is self._sem_poison
    self.nc.clear_and_free_semaphores(list(self.sems.allocated().values()))
    self.nc.all_engine_barrier()


tile.TileContext._drain_and_barrier = _drain_and_barrier


# ---------------------------------------------------------------------------
# Per-core sharding layout (host side)
# ---------------------------------------------------------------------------
def core_layout(g):
    """For group index g (0..3): (qtiles sorted, perm) with the core's query
    tiles at positions QPOS and every tile's causal prefix placed before it."""
    qtiles = sorted([g, 7 - g, 8 + g, 15 - g])
    posmap = dict(zip(QPOS, qtiles))
    rest = iter([t for t in range(TT) if t not in qtiles])
    perm = [posmap[p] if p in posmap else next(rest) for p in range(TT)]
    # causal validity: tiles <= qtiles[j] all sit at positions <= QPOS[j]
    for j, a in enumerate(qtiles):
        assert set(range(a + 1)) <= set(perm[: QPOS[j] + 1]), (g, j, perm)
    return qtiles, perm


def core_masks(qtiles, perm):
    """masks[kp] = causal mask of k-position kp against query tile j=kp//4
    (the first in-suffix block - across all core layouts the only block
    that is ever not all-ones)."""
    masks = np.zeros((TT, 128, 128), dtype=BF16)
    for kp in range(TT):
        tk = perm[kp] * 128 + np.arange(128)[:, None]
        a = qtiles[kp // 4]
        tq = a * 128 + np.arange(128)[None, :]
        masks[kp] = (tk <= tq).astype(BF16)
    return masks


def pack_dr(W):
    """[K, N] fp32 -> DoubleRow-paired fp8 [K/256, 128, 2, N], pre-scaled.
    Logical k = 256*kt + 128*r + p."""
    K, N = W.shape
    Wp = (np.asarray(W, np.float32) * WS).reshape(K // 256, 2, 128, N)
    return np.ascontiguousarray(Wp.transpose(0, 2, 1, 3)).astype(NP_F8)


# ---------------------------------------------------------------------------
# The Bass program (identical for all 8 cores)
# ---------------------------------------------------------------------------
def build_program():
    nc = bacc.Bacc("TRN2")

    d_x = nc.dram_tensor("x_perm", [T, C], F32, kind="ExternalInput")
    d_xob = nc.dram_tensor("x_own_b", [NQ, C], F32, kind="ExternalInput")
    d_masks = nc.dram_tensor("masks", [TT, 128, 128], BF, kind="ExternalInput")
    d_wq = nc.dram_tensor("wq", [KT, 128, 2, C], F8, kind="ExternalInput")
    d_wk = nc.dram_tensor("wk", [KT, 128, 2, C], F8, kind="ExternalInput")
    d_wv = nc.dram_tensor("wv", [KT, 128, 2, C], F8, kind="ExternalInput")
    d_wp = nc.dram_tensor("wp", [C, C], BF, kind="ExternalInput")
    d_wfc = nc.dram_tensor("wfc", [C, DFF], BF, kind="ExternalInput")
    d_wfc2 = nc.dram_tensor("wfc2", [DFF, C], BF, kind="ExternalInput")
    d_bq = nc.dram_tensor("bq2d", [128, CT], F32, kind="ExternalInput")
    d_bk = nc.dram_tensor("bk2d", [128, CT], F32, kind="ExternalInput")
    d_bv = nc.dram_tensor("bv_bc", [128, C], F32, kind="ExternalInput")
    d_bfc = nc.dram_tensor("bfc_bc", [128, DFF], F32, kind="ExternalInput")
    d_bfc2 = nc.dram_tensor("bfc2_bc", [128, C], F32, kind="ExternalInput")
    d_l1g = nc.dram_tensor("ln1g2d", [128, CT], F32, kind="ExternalInput")
    d_l1b = nc.dram_tensor("ln1b2d", [128, CT], F32, kind="ExternalInput")
    d_l2g = nc.dram_tensor("ln2g2d", [128, CT], F32, kind="ExternalInput")
    d_l2b = nc.dram_tensor("ln2b2d", [128, CT], F32, kind="ExternalInput")
    d_ident = nc.dram_tensor("ident", [128, 128], BF, kind="ExternalInput")
    d_out = nc.dram_tensor("out", [NQ, C], F32, kind="ExternalOutput")

    with tile.TileContext(nc) as tc:
        _body(nc, tc, locals())
    nc.compile()
    return nc


def _ln_tile(nc, pool, x_ap, eps):
    """LayerNorm stats for one [128, C] fp32 tile -> (mu, rstd) [128,1]."""
    stats = pool.tile([128, 3, 6], F32, tag="bnstats", name="bnstats")
    xg = x_ap.rearrange("p (a b) -> p a b", b=256)
    for a in range(3):
        nc.vector.bn_stats(out=stats[:, a, :], in_=xg[:, a, :])
    mv = pool.tile([128, 2], F32, tag="bnaggr", name="bnaggr")
    nc.vector.bn_aggr(out=mv[:], in_=stats[:])
    sd = pool.tile([128, 1], F32, tag="sd", name="sd")
    nc.scalar.activation(out=sd[:], in_=mv[:, 1:2], func=AF.Sqrt, bias=eps[:])
    rstd = pool.tile([128, 1], F32, tag="rstd", name="rstd")
    nc.vector.reciprocal(out=rstd[:], in_=sd[:])
    return mv[:, 0:1], rstd


def _body(nc, tc, d):
    def pool(name, **kw):
        return tc.tile_pool(name=name, **kw)

    with (
        pool("const", bufs=1) as constp,
        pool("persist", bufs=1) as pers,
        pool("small", bufs=6) as small,
    ):
        # ---- constants (tiny DMAs first) --------------------------------
        ident = constp.tile([128, 128], BF)
        nc.sync.dma_start(ident[:], d["d_ident"][:])
        eps = constp.tile([128, 1], F32)
        nc.vector.memset(eps[:], 1e-5)
        l1g = constp.tile([128, CT], F32)
        nc.sync.dma_start(l1g[:], d["d_l1g"][:])
        l1b = constp.tile([128, CT], F32)
        nc.sync.dma_start(l1b[:], d["d_l1b"][:])
        l2g = constp.tile([128, CT], F32)
        nc.sync.dma_start(l2g[:], d["d_l2g"][:])
        l2b = constp.tile([128, CT], F32)
        nc.sync.dma_start(l2b[:], d["d_l2b"][:])
        bq = constp.tile([128, CT], F32)
        nc.sync.dma_start(bq[:], d["d_bq"][:])
        bk = constp.tile([128, CT], F32)
        nc.sync.dma_start(bk[:], d["d_bk"][:])
        bv_bc = constp.tile([128, C], F32)
        nc.sync.dma_start(bv_bc[:], d["d_bv"][:])
        bfc2_bc = constp.tile([128, C], F32)
        nc.sync.dma_start(bfc2_bc[:], d["d_bfc2"][:])
        ones64 = constp.tile([1, 64], F32)
        nc.vector.memset(ones64[:], 1.0)

        # ---- persistent activations -----------------------------------
        x_own = [pers.tile([128, C], F32, tag=f"xo{j}", name=f"xo{j}") for j in range(4)]
        qT = [pers.tile([128, NQ], BF, tag=f"qT{t}", name=f"qT{t}") for t in range(CT)]
        yT = [pers.tile([128, NQ], BF, tag=f"yT{t}", name=f"yT{t}") for t in range(CT)]

        with pool("attn_sb", bufs=1) as attnp:
            kT = [attnp.tile([128, T], BF, tag=f"kT{t}", name=f"kT{t}") for t in range(CT)]
            V = [attnp.tile([128, H * (HD + 1)], BF, tag=f"V{t}", name=f"V{t}") for t in range(TT)]
            masks = [attnp.tile([128, 128], BF, tag=f"m{t}", name=f"m{t}") for t in range(TT)]

            # ======== phase 1: LN1 + transpose to feature-major fp8 ========
            with (
                pool("ph1", bufs=1) as ph1p,
                pool("ph1s", bufs=4) as ph1s,
            ):
                xnT8 = [ph1p.tile([128, 2, T], F8, tag=f"xnT{t}", name=f"xnT{t}") for t in range(KT)]
                xnTq8 = [ph1p.tile([128, 2, NQ], F8, tag=f"xnTq{t}", name=f"xnTq{t}") for t in range(KT)]
                with pool("ph1t", bufs=1, space="PSUM") as ph1t:
                    for ttg in range(4):
                        ptb = [
                            ph1t.tile([128, 512], BF, tag=f"ptb{t}", name=f"ptb{t}")
                            for t in range(CT)
                        ]
                        for ti in range(4):
                            tt = ttg * 4 + ti
                            xt = ph1s.tile([128, C], F32, tag="xt", name="xt")
                            nc.scalar.dma_start(xt[:], d["d_x"][tt * 128 : (tt + 1) * 128, :])
                            mu, rstd = _ln_tile(nc, small, xt[:], eps)
                            xn = ph1s.tile([128, C], BF, tag="xn", name="xn")
                            nc.vector.tensor_scalar(
                                out=xn[:], in0=xt[:], scalar1=mu, scalar2=rstd[:],
                                op0=ALU.subtract, op1=ALU.mult,
                            )
                            for ct in range(CT):
                                nc.tensor.transpose(
                                    ptb[ct][:, ti * 128 : (ti + 1) * 128],
                                    xn[:, ct * 128 : (ct + 1) * 128], ident[:],
                                )
                        for ct in range(CT):
                            kt, r = ct // 2, ct % 2
                            nc.scalar.activation(
                                out=xnT8[kt][:, r, ttg * 512 : (ttg + 1) * 512],
                                in_=ptb[ct][:], func=AF.Identity,
                                scale=l1g[:, ct : ct + 1], bias=l1b[:, ct : ct + 1],
                            )
                            # own q-tile of this group sits at ti == 3
                            nc.scalar.activation(
                                out=xnTq8[kt][:, r, ttg * 128 : (ttg + 1) * 128],
                                in_=ptb[ct][:, 384:512], func=AF.Identity,
                                scale=l1g[:, ct : ct + 1], bias=l1b[:, ct : ct + 1],
                            )

                # ======== phase 2: Q^T, K^T, V (fp8 DoubleRow) ========
                with (
                    pool("wqkv", bufs=1) as wp_,
                    pool("ph2ps", bufs=3, space="PSUM") as ph2ps,
                    pool("ph2pv", bufs=2, space="PSUM") as ph2pv,
                ):
                    wq8 = [wp_.tile([128, 2, C], F8, tag=f"wq{t}", name=f"wq{t}") for t in range(KT)]
                    wk8 = [wp_.tile([128, 2, C], F8, tag=f"wk{t}", name=f"wk{t}") for t in range(KT)]
                    wv8 = [wp_.tile([128, 2, C], F8, tag=f"wv{t}", name=f"wv{t}") for t in range(KT)]
                    for t in range(KT):
                        nc.sync.dma_start(wk8[t][:], d["d_wk"][t, :, :, :])
                    for t in range(KT):
                        nc.sync.dma_start(wv8[t][:], d["d_wv"][t, :, :, :])
                    for t in range(KT):
                        nc.sync.dma_start(wq8[t][:], d["d_wq"][t, :, :, :])
                    # masks on the gpsimd queue, out of the weight path
                    for t in range(TT):
                        nc.gpsimd.dma_start(masks[t][:], d["d_masks"][t, :, :])

                    # kT chunk nn and V tiles 4nn..4nn+3 depend only on
                    # token-group nn of xnT -> overlap with phase 1 tail
                    for nn in range(4):
                        for f in range(CT):
                            ps = ph2ps.tile([128, 512], F32, tag="pqk", name="pk")
                            for kt in range(KT):
                                nc.tensor.matmul(
                                    ps[:], wk8[kt][:, :, f * 128 : (f + 1) * 128],
                                    xnT8[kt][:, :, nn * 512 : (nn + 1) * 512],
                                    start=(kt == 0), stop=(kt == KT - 1),
                                    perf_mode=DR,
                                )
                            nc.vector.tensor_scalar(
                                out=kT[f][:, nn * 512 : (nn + 1) * 512],
                                in0=ps[:], scalar1=bk[:, f : f + 1],
                                scalar2=None, op0=ALU.add,
                            )
                        for tt in range(nn * 4, nn * 4 + 4):
                            pv = ph2pv.tile([128, C], F32, tag="pv", name="pv")
                            for lo, hi in ((0, 512), (512, 768)):
                                for kt in range(KT):
                                    nc.tensor.matmul(
                                        pv[:, lo:hi],
                                        xnT8[kt][:, :, tt * 128 : (tt + 1) * 128],
                                        wv8[kt][:, :, lo:hi],
                                        start=(kt == 0), stop=(kt == KT - 1),
                                        perf_mode=DR,
                                    )
                            vt = V[tt][:].rearrange("p (h e) -> p h e", e=HD + 1)
                            # ones column = WS so the fp8 weight scale cancels
                            # between softmax numerator and denominator
                            nc.vector.memset(vt[:, :, HD : HD + 1], WS)
                            pvh = pv[:].rearrange("p (h e) -> p h e", e=HD)
                            nc.vector.tensor_tensor(
                                out=vt[:, :, 0:HD], in0=pvh[:],
                                in1=bv_bc[:].rearrange("p (h e) -> p h e", e=HD),
                                op=ALU.add,
                            )
                    # qT[f] [128, 512] = (Wq[:, f].T @ xnTq) + bq
                    for f in range(CT):
                        ps = ph2ps.tile([128, NQ], F32, tag="pqk", name="pq")
                        for kt in range(KT):
                            nc.tensor.matmul(
                                ps[:], wq8[kt][:, :, f * 128 : (f + 1) * 128],
                                xnTq8[kt][:], start=(kt == 0), stop=(kt == KT - 1),
                                perf_mode=DR,
                            )
                        nc.vector.tensor_scalar(
                            out=qT[f][:], in0=ps[:], scalar1=bq[:, f : f + 1],
                            scalar2=None, op0=ALU.add,
                        )

            # ======== phase 3: attention (bf16, 4-head pipeline) ========
            with (
                pool("ph3", bufs=8) as ph3s,
                pool("ph3ps", bufs=3, space="PSUM") as ph3ps,
                pool("ph3pa", bufs=1, space="PSUM") as ph3pa,
            ):
                for hg in range(H // 4):
                    hs = [hg * 4 + i for i in range(4)]
                    pavs = {
                        h: ph3pa.tile(
                            [128, NQ], F32, tag=f"pav{h % 4}", name=f"pav{h % 4}"
                        )
                        for h in hs
                    }
                    for kp in range(TT):
                        cs = 128 * (kp // 4)
                        psbs = {}
                        for h in hs:
                            ro = (h % 2) * 64
                            ps = ph3ps.tile([128, NQ], F32, tag="ps", name="ps")
                            nc.tensor.matmul(
                                ps[:, cs:NQ],
                                kT[h // 2][ro : ro + 64, kp * 128 : (kp + 1) * 128],
                                qT[h // 2][ro : ro + 64, cs:NQ],
                            )
                            p_sb = ph3s.tile([128, NQ], BF, tag="p_sb", name="p_sb")
                            # q,k both carry the 32x fp8 weight scale
                            nc.scalar.activation(
                                out=p_sb[:, cs:NQ], in_=ps[:, cs:NQ],
                                func=AF.Exp, scale=0.125 / (WS * WS),
                            )
                            # only the first in-suffix 128-col block is ever
                            # not all-ones (across every core layout)
                            nc.vector.tensor_mul(
                                p_sb[:, cs : cs + 128], p_sb[:, cs : cs + 128],
                                masks[kp][:],
                            )
                            psbs[h] = p_sb
                        for h in hs:
                            nc.tensor.matmul(
                                pavs[h][0 : HD + 1, cs:NQ],
                                V[kp][:, h * (HD + 1) : (h + 1) * (HD + 1)],
                                psbs[h][:, cs:NQ],
                                start=(kp == 0), stop=(kp == TT - 1),
                                skip_group_check=True,
                            )
                    for h in hs:
                        ro = (h % 2) * 64
                        pav = pavs[h]
                        den = ph3s.tile([1, NQ], F32, tag="den", name="den")
                        nc.scalar.copy(den[:], pav[HD : HD + 1, :])
                        rbp = ph3ps.tile([64, NQ], F32, tag="rbp", name="rbp", bufs=1)
                        nc.tensor.matmul(rbp[:], ones64[:], den[:])
                        rb = ph3s.tile([64, NQ], F32, tag="rb", name="rb")
                        nc.vector.reciprocal(out=rb[:], in_=rbp[:])
                        nc.vector.tensor_tensor(
                            out=yT[h // 2][ro : ro + 64, :],
                            in0=pav[0:HD, :], in1=rb[:], op=ALU.mult,
                        )

        # ======== phase 4: proj + residual + LN2 ========
        with pool("mlp_sb", bufs=1) as mlpp:
            x2 = [mlpp.tile([128, C], F32, tag=f"x2{j}", name=f"x2{j}") for j in range(4)]
            xn2T = [mlpp.tile([128, NQ], BF, tag=f"xn2T{t}", name=f"xn2T{t}") for t in range(CT)]
            hT = [mlpp.tile([128, NQ], BF, tag=f"hT{t}", name=f"hT{t}") for t in range(FT)]
            with (
                pool("mlp1", bufs=1) as m1p,
                pool("mlp1s", bufs=3) as m1s,
            ):
                wp = [m1p.tile([128, C], BF, tag=f"wp{t}", name=f"wp{t}") for t in range(CT)]
                wfc = [m1p.tile([128, DFF], BF, tag=f"wfc{t}", name=f"wfc{t}") for t in range(CT)]
                bfc_bc = m1p.tile([128, DFF], F32, tag="bfcbc", name="bfcbc")
                hh = [m1p.tile([128, DFF], BF, tag=f"hh{j}", name=f"hh{j}") for j in range(4)]
                for t in range(CT):
                    nc.sync.dma_start(wp[t][:], d["d_wp"][t * 128 : (t + 1) * 128, :])
                for t in range(CT):
                    nc.sync.dma_start(wfc[t][:], d["d_wfc"][t * 128 : (t + 1) * 128, :])
                nc.sync.dma_start(bfc_bc[:], d["d_bfc"][:])
                for j in range(4):
                    nc.scalar.dma_start(x_own[j][:], d["d_xob"][j * 128 : (j + 1) * 128, :])

                with (
                    pool("ph4p", bufs=2, space="PSUM") as ph4p,
                    pool("ph4t", bufs=4, space="PSUM") as ph4t,
                ):
                    for qt in range(4):
                        pp = ph4p.tile([128, C], F32, tag="pp", name="pp")
                        for lo, hi in ((0, 512), (512, 768)):
                            for ct in range(CT):
                                nc.tensor.matmul(
                                    pp[:, lo:hi],
                                    yT[ct][:, qt * 128 : (qt + 1) * 128],
                                    wp[ct][:, lo:hi],
                                    start=(ct == 0), stop=(ct == CT - 1),
                                )
                        nc.vector.tensor_add(x2[qt][:], pp[:], x_own[qt][:])
                        mu, rstd = _ln_tile(nc, small, x2[qt][:], eps)
                        xn2 = m1s.tile([128, C], BF, tag="xn2", name="xn2")
                        nc.vector.tensor_scalar(
                            out=xn2[:], in0=x2[qt][:], scalar1=mu, scalar2=rstd[:],
                            op0=ALU.subtract, op1=ALU.mult,
                        )
                        for ct in range(CT):
                            pt = ph4t.tile([128, 128], BF, tag="pt4", name="pt4")
                            nc.tensor.transpose(
                                pt[:], xn2[:, ct * 128 : (ct + 1) * 128], ident[:]
                            )
                            nc.scalar.activation(
                                out=xn2T[ct][:, qt * 128 : (qt + 1) * 128],
                                in_=pt[:], func=AF.Identity,
                                scale=l2g[:, ct : ct + 1], bias=l2b[:, ct : ct + 1],
                            )

                # ======== phase 5: fc + gelu ========
                with pool("ph5p", bufs=2, space="PSUM") as ph5p:
                    for qt in range(4):
                        for nn in range(6):
                            sl = slice(nn * 512, (nn + 1) * 512)
                            ph_ = ph5p.tile([128, 512], F32, tag="ph5", name="ph5")
                            for ct in range(CT):
                                nc.tensor.matmul(
                                    ph_[:],
                                    xn2T[ct][:, qt * 128 : (qt + 1) * 128],
                                    wfc[ct][:, sl],
                                    start=(ct == 0), stop=(ct == CT - 1),
                                )
                            tmp = m1s.tile([128, 512], BF, tag="pregelu", name="pregelu")
                            nc.vector.tensor_add(tmp[:], ph_[:], bfc_bc[:, sl])
                            nc.scalar.activation(
                                out=hh[qt][:, sl], in_=tmp[:], func=AF.Gelu_apprx_tanh,
                            )

                # ======== phase 6: transpose h ========
                with pool("ph6t", bufs=2, space="PSUM") as ph6t:
                    for fc in range(FT):
                        ptb = ph6t.tile([128, 512], BF, tag="pt6", name="pt6")
                        for qt in range(4):
                            nc.tensor.transpose(
                                ptb[:, qt * 128 : (qt + 1) * 128],
                                hh[qt][:, fc * 128 : (fc + 1) * 128], ident[:],
                            )
                        nc.scalar.copy(hT[fc][:], ptb[:])

            # ======== phase 7: fc2 + residual + out ========
            with (
                pool("mlp2", bufs=1) as m2p,
                pool("mlp2s", bufs=3) as m2s,
                pool("ph7p", bufs=2, space="PSUM") as ph7p,
            ):
                wfc2 = [m2p.tile([128, C], BF, tag=f"wfc2{t}", name=f"wfc2{t}") for t in range(FT)]
                for t in range(FT):
                    nc.sync.dma_start(wfc2[t][:], d["d_wfc2"][t * 128 : (t + 1) * 128, :])
                for qt in range(4):
                    po = ph7p.tile([128, C], F32, tag="po", name="po")
                    for lo, hi in ((0, 512), (512, 768)):
                        for kt in range(FT):
                            nc.tensor.matmul(
                                po[:, lo:hi],
                                hT[kt][:, qt * 128 : (qt + 1) * 128],
                                wfc2[kt][:, lo:hi],
                                start=(kt == 0), stop=(kt == FT - 1),
                            )
                    t1 = m2s.tile([128, C], F32, tag="t1", name="t1")
                    nc.vector.tensor_add(t1[:], po[:], bfc2_bc[:])
                    ot = m2s.tile([128, C], F32, tag="ot", name="ot")
                    nc.vector.tensor_add(ot[:], t1[:], x2[qt][:])
                    nc.sync.dma_start(
                        d["d_out"][qt * 128 : (qt + 1) * 128, :], ot[:]
                    )


# ---------------------------------------------------------------------------
# Host-side wrapper
# ---------------------------------------------------------------------------
_PROGRAM = None


def _get_program():
    global _PROGRAM
    if _PROGRAM is None:
        _PROGRAM = build_program()
    return _PROGRAM


def make_in_maps(x, ln1_g, ln1_b, W_attn, b_attn, W_proj, b_proj,
                 ln2_g, ln2_b, W_fc, b_fc, W_fc2, b_fc2):
    x = np.asarray(x, np.float32)
    shared = {
        "wq": pack_dr(W_attn[:, 0:C]),
        "wk": pack_dr(W_attn[:, C : 2 * C]),
        "wv": pack_dr(W_attn[:, 2 * C : 3 * C]),
        "wp": np.asarray(W_proj, BF16),
        "wfc": np.asarray(W_fc, BF16),
        "wfc2": np.asarray(W_fc2, BF16),
        # q/k/v biases ride the 32x weight scale
        "bq2d": np.ascontiguousarray(
            np.asarray(b_attn[0:C], np.float32).reshape(CT, 128).T * WS),
        "bk2d": np.ascontiguousarray(
            np.asarray(b_attn[C : 2 * C], np.float32).reshape(CT, 128).T * WS),
        "bv_bc": np.broadcast_to(
            np.asarray(b_attn[2 * C : 3 * C], np.float32) * WS, (128, C)).copy(),
        "bfc_bc": np.broadcast_to(
            np.asarray(b_fc, np.float32), (128, DFF)).copy(),
        "bfc2_bc": np.broadcast_to(
            np.asarray(b_fc2, np.float32), (128, C)).copy(),
        "ln1g2d": np.ascontiguousarray(
            np.asarray(ln1_g, np.float32).reshape(CT, 128).T),
        "ln1b2d": np.ascontiguousarray(
            np.asarray(ln1_b, np.float32).reshape(CT, 128).T),
        "ln2g2d": np.ascontiguousarray(
            np.asarray(ln2_g, np.float32).reshape(CT, 128).T),
        "ln2b2d": np.ascontiguousarray(
            np.asarray(ln2_b, np.float32).reshape(CT, 128).T),
        "ident": np.eye(128, dtype=BF16),
    }
    bp = np.asarray(b_proj, np.float32)
    in_maps, layouts = [], []
    for core in range(8):
        b, g = core // 4, core % 4
        qtiles, perm = core_layout(g)
        idx = np.concatenate([np.arange(t * 128, (t + 1) * 128) for t in perm])
        own = np.concatenate([np.arange(t * 128, (t + 1) * 128) for t in qtiles])
        m = dict(shared)
        m["x_perm"] = np.ascontiguousarray(x[b][idx])
        m["x_own_b"] = np.ascontiguousarray(x[b][own] + bp)
        m["masks"] = core_masks(qtiles, perm)
        in_maps.append(m)
        layouts.append((b, own))
    return in_maps, layouts


def unshard(results, layouts):
    out = np.empty((B, T, C), np.float32)
    for r, (b, own) in zip(results, layouts):
        out[b][own] = r["out"]
    return out


def kernel(**inputs):
    from concourse.bass_utils import run_bass_kernel_spmd

    nc = _get_program()
    in_maps, layouts = make_in_maps(**inputs)
    res = run_bass_kernel_spmd(nc, in_maps, core_ids=list(range(8)))
    return unshard(res.results, layouts)


# revision 17
# speedup vs baseline: 1.3411x; 1.1692x over previous
"""GPT-2 block (B=2, T=2048, C=768, H=12) on 8 Trainium2 NeuronCores.

Sharding: data-parallel over batch (2) x 4-way query-tile split per batch.
Each core computes K/V for its full batch (avoids on-chip collectives,
whose latency floor exceeds the redundant compute) and runs attention +
MLP for 4 of the 16 query tiles, interleaved {g, 7-g, 8+g, 15-g} so the
causal-attention work is identical across cores.

The SPMD program is uniform across cores: per-core differences are pushed
into the data via a k-tile permutation of the sequence (each core's query
tiles sit at fixed positions {3,7,11,15}; every tile's causal prefix is
placed before it) plus per-core causal masks.

Layouts: activations enter matmuls feature-major (xnT [C,T]) so QKV needs
no transposes; attention scores are computed transposed (S^T [k,q]) so
exp(S^T) is directly the stationary operand of the A*V matmul, and a ones
column appended to V produces the softmax denominator in the same matmul.

Precision: weights and matmul activations are fp8 e4m3 with DoubleRow
matmuls (2 K-rows/cycle, K-tiles of 256). Weights are pre-scaled by 32 on
the host to stay in the fp8 normal range; the scale folds into the exp()
argument for attention (q,k both 32x -> scale/1024), into the V ones
column (=32 so softmax numerator/denominator cancel), and into one cheap
descale per MLP/proj output. Attention S/AV matmuls stay bf16.
"""

import sys

sys.path.insert(0, "/opt/trn_rl_repo")

import numpy as np
import ml_dtypes

import bass_rust
import concourse.bass as bass
import concourse.bacc as bacc
import concourse.tile as tile
from concourse import mybir
from concourse.vector_clock import ScopedClock

BF16 = ml_dtypes.bfloat16
F32 = mybir.dt.float32
BF = mybir.dt.bfloat16
F8 = mybir.dt.float8e4
NP_F8 = mybir.dt.np(F8)

B, T, C, H = 2, 2048, 768, 12
HD = C // H  # 64
DFF = 4 * C  # 3072
TT = T // 128  # 16 token tiles
CT = C // 128  # 6 feature tiles
KT = C // 256  # 3 DoubleRow k-tiles over C
KT2 = DFF // 256  # 12 DoubleRow k-tiles over DFF
FT = DFF // 128  # 24
QPOS = (3, 7, 11, 15)  # fixed positions of this core's query tiles
NQ = 512  # queries per core
WS = 32.0  # fp8 weight pre-scale
AF = mybir.ActivationFunctionType
ALU = mybir.AluOpType
DR = mybir.MatmulPerfMode.DoubleRow

# ---------------------------------------------------------------------------
# Tile exit-drain fix: the final SP drain carries one wait per live logical
# processor, but TRN2 ISA instructions hold at most 1 embedded sync wait in
# this toolchain. Split the waits across a chain of SP drains.
# ---------------------------------------------------------------------------
_MAX_WAITS = 1


def _drain_and_barrier(self, tick_clock, wait_clock):
    drain_inst = self.nc.sync.drain()
    wait_clock.add_sem_waits(
        drain_inst.ins, ScopedClock({None: tick_clock.global_clock})
    )
    si = drain_inst.ins.sync_info
    if si is not None and len(si.on_wait) > _MAX_WAITS:
        waits = list(si.on_wait)
        drain_inst.ins.sync_info = bass_rust.SyncInfo(
            on_wait=waits[:_MAX_WAITS], on_update=list(si.on_update)
        )
        rest = waits[_MAX_WAITS:]
        for i in range(0, len(rest), _MAX_WAITS):
            extra = self.nc.sync.drain()
            extra.ins.sync_info = bass_rust.SyncInfo(
                on_wait=rest[i : i + _MAX_WAITS], on_update=[]
            )
    self.nc.all_engine_barrier()
    assert self.sems is not None
    popped = self.nc._tile_sem_poison_stack.pop()
    assert popped is self._sem_poison
    self.nc.clear_and_free_semaphores(list(self.sems.allocated().values()))
    self.nc.all_engine_barrier()


tile.TileContext._drain_and_barrier = _drain_and_barrier


# ---------------------------------------------------------------------------
# Per-core sharding layout (host side)
# ---------------------------------------------------------------------------
def core_layout(g):
    """For group index g (0..3): (qtiles sorted, perm) with the core's query
    tiles at positions QPOS and every tile's causal prefix placed before it."""
    qtiles = sorted([g, 7 - g, 8 + g, 15 - g])
    posmap = dict(zip(QPOS, qtiles))
    rest = iter([t for t in range(TT) if t not in qtiles])
    perm = [posmap[p] if p in posmap else next(rest) for p in range(TT)]
    # causal validity: tiles <= qtiles[j] all sit at positions <= QPOS[j]
    for j, a in enumerate(qtiles):
        assert set(range(a + 1)) <= set(perm[: QPOS[j] + 1]), (g, j, perm)
    return qtiles, perm


def core_masks(qtiles, perm):
    """masks[kp] = causal mask of k-position kp against query tile j=kp//4
    (the first in-suffix block - across all core layouts the only block
    that is ever not all-ones)."""
    masks = np.zeros((TT, 128, 128), dtype=BF16)
    for kp in range(TT):
        tk = perm[kp] * 128 + np.arange(128)[:, None]
        a = qtiles[kp // 4]
        tq = a * 128 + np.arange(128)[None, :]
        masks[kp] = (tk <= tq).astype(BF16)
    return masks


def pack_dr(W):
    """[K, N] fp32 -> DoubleRow-paired fp8 [K/256, 128, 2, N], pre-scaled.
    Logical k = 256*kt + 128*r + p."""
    K, N = W.shape
    Wp = (np.asarray(W, np.float32) * WS).reshape(K // 256, 2, 128, N)
    return np.ascontiguousarray(Wp.transpose(0, 2, 1, 3)).astype(NP_F8)


# ---------------------------------------------------------------------------
# The Bass program (identical for all 8 cores)
# ---------------------------------------------------------------------------
def build_program():
    nc = bacc.Bacc("TRN2")

    d_x = nc.dram_tensor("x_perm", [T, C], F32, kind="ExternalInput")
    d_xob = nc.dram_tensor("x_own_b", [NQ, C], F32, kind="ExternalInput")
    d_masks = nc.dram_tensor("masks", [TT, 128, 128], BF, kind="ExternalInput")
    d_wq = nc.dram_tensor("wq", [KT, 128, 2, C], F8, kind="ExternalInput")
    d_wk = nc.dram_tensor("wk", [KT, 128, 2, C], F8, kind="ExternalInput")
    d_wv = nc.dram_tensor("wv", [KT, 128, 2, C], F8, kind="ExternalInput")
    d_wp = nc.dram_tensor("wp", [C, C], BF, kind="ExternalInput")
    d_wfc = nc.dram_tensor("wfc", [C, DFF], BF, kind="ExternalInput")
    d_wfc2 = nc.dram_tensor("wfc2", [DFF, C], BF, kind="ExternalInput")
    d_bq = nc.dram_tensor("bq2d", [128, CT], F32, kind="ExternalInput")
    d_bk = nc.dram_tensor("bk2d", [128, CT], F32, kind="ExternalInput")
    d_bv = nc.dram_tensor("bv_bc", [128, C], F32, kind="ExternalInput")
    d_bfc = nc.dram_tensor("bfc_bc", [128, DFF], F32, kind="ExternalInput")
    d_bfc2 = nc.dram_tensor("bfc2_bc", [128, C], F32, kind="ExternalInput")
    d_l1g = nc.dram_tensor("ln1g2d", [128, CT], F32, kind="ExternalInput")
    d_l1b = nc.dram_tensor("ln1b2d", [128, CT], F32, kind="ExternalInput")
    d_l2g = nc.dram_tensor("ln2g2d", [128, CT], F32, kind="ExternalInput")
    d_l2b = nc.dram_tensor("ln2b2d", [128, CT], F32, kind="ExternalInput")
    d_ident = nc.dram_tensor("ident", [128, 128], BF, kind="ExternalInput")
    d_out = nc.dram_tensor("out", [NQ, C], F32, kind="ExternalOutput")

    with tile.TileContext(nc) as tc:
        _body(nc, tc, locals())
    nc.compile()
    return nc


def _ln_tile(nc, pool, x_ap, eps):
    """LayerNorm stats for one [128, C] fp32 tile -> (mu, rstd) [128,1]."""
    stats = pool.tile([128, 3, 6], F32, tag="bnstats", name="bnstats")
    xg = x_ap.rearrange("p (a b) -> p a b", b=256)
    for a in range(3):
        nc.vector.bn_stats(out=stats[:, a, :], in_=xg[:, a, :])
    mv = pool.tile([128, 2], F32, tag="bnaggr", name="bnaggr")
    nc.vector.bn_aggr(out=mv[:], in_=stats[:])
    sd = pool.tile([128, 1], F32, tag="sd", name="sd")
    nc.scalar.activation(out=sd[:], in_=mv[:, 1:2], func=AF.Sqrt, bias=eps[:])
    rstd = pool.tile([128, 1], F32, tag="rstd", name="rstd")
    nc.vector.reciprocal(out=rstd[:], in_=sd[:])
    return mv[:, 0:1], rstd


def _body(nc, tc, d):
    def pool(name, **kw):
        return tc.tile_pool(name=name, **kw)

    with (
        pool("const", bufs=1) as constp,
        pool("persist", bufs=1) as pers,
        pool("small", bufs=6) as small,
    ):
        # ---- constants (tiny DMAs first) --------------------------------
        ident = constp.tile([128, 128], BF)
        nc.sync.dma_start(ident[:], d["d_ident"][:])
        eps = constp.tile([128, 1], F32)
        nc.vector.memset(eps[:], 1e-5)
        l1g = constp.tile([128, CT], F32)
        nc.sync.dma_start(l1g[:], d["d_l1g"][:])
        l1b = constp.tile([128, CT], F32)
        nc.sync.dma_start(l1b[:], d["d_l1b"][:])
        l2g = constp.tile([128, CT], F32)
        nc.sync.dma_start(l2g[:], d["d_l2g"][:])
        l2b = constp.tile([128, CT], F32)
        nc.sync.dma_start(l2b[:], d["d_l2b"][:])
        bq = constp.tile([128, CT], F32)
        nc.sync.dma_start(bq[:], d["d_bq"][:])
        bk = constp.tile([128, CT], F32)
        nc.sync.dma_start(bk[:], d["d_bk"][:])
        bv_bc = constp.tile([128, C], F32)
        nc.sync.dma_start(bv_bc[:], d["d_bv"][:])
        bfc2_bc = constp.tile([128, C], F32)
        nc.sync.dma_start(bfc2_bc[:], d["d_bfc2"][:])
        ones64 = constp.tile([1, 64], F32)
        nc.vector.memset(ones64[:], 1.0)

        # ---- persistent activations -----------------------------------
        x_own = [pers.tile([128, C], F32, tag=f"xo{j}", name=f"xo{j}") for j in range(4)]
        qT = [pers.tile([128, NQ], BF, tag=f"qT{t}", name=f"qT{t}") for t in range(CT)]
        yT = [pers.tile([128, NQ], BF, tag=f"yT{t}", name=f"yT{t}") for t in range(CT)]

        with pool("attn_sb", bufs=1) as attnp:
            kT = [attnp.tile([128, T], BF, tag=f"kT{t}", name=f"kT{t}") for t in range(CT)]
            V = [attnp.tile([128, H * (HD + 1)], BF, tag=f"V{t}", name=f"V{t}") for t in range(TT)]
            masks = [attnp.tile([128, 128], BF, tag=f"m{t}", name=f"m{t}") for t in range(TT)]

            # ======== phase 1: LN1 + transpose to feature-major fp8 ========
            with (
                pool("ph1", bufs=1) as ph1p,
                pool("ph1s", bufs=4) as ph1s,
            ):
                xnT8 = [ph1p.tile([128, 2, T], F8, tag=f"xnT{t}", name=f"xnT{t}") for t in range(KT)]
                xnTq8 = [ph1p.tile([128, 2, NQ], F8, tag=f"xnTq{t}", name=f"xnTq{t}") for t in range(KT)]
                with pool("ph1t", bufs=1, space="PSUM") as ph1t:
                    for ttg in range(4):
                        ptb = [
                            ph1t.tile([128, 512], BF, tag=f"ptb{t}", name=f"ptb{t}")
                            for t in range(CT)
                        ]
                        for ti in range(4):
                            tt = ttg * 4 + ti
                            xt = ph1s.tile([128, C], F32, tag="xt", name="xt")
                            nc.scalar.dma_start(xt[:], d["d_x"][tt * 128 : (tt + 1) * 128, :])
                            mu, rstd = _ln_tile(nc, small, xt[:], eps)
                            xn = ph1s.tile([128, C], BF, tag="xn", name="xn")
                            nc.vector.tensor_scalar(
                                out=xn[:], in0=xt[:], scalar1=mu, scalar2=rstd[:],
                                op0=ALU.subtract, op1=ALU.mult,
                            )
                            for ct in range(CT):
                                nc.tensor.transpose(
                                    ptb[ct][:, ti * 128 : (ti + 1) * 128],
                                    xn[:, ct * 128 : (ct + 1) * 128], ident[:],
                                )
                        for ct in range(CT):
                            kt, r = ct // 2, ct % 2
                            nc.scalar.activation(
                                out=xnT8[kt][:, r, ttg * 512 : (ttg + 1) * 512],
                                in_=ptb[ct][:], func=AF.Identity,
                                scale=l1g[:, ct : ct + 1], bias=l1b[:, ct : ct + 1],
                            )
                            # own q-tile of this group sits at ti == 3
                            nc.scalar.activation(
                                out=xnTq8[kt][:, r, ttg * 128 : (ttg + 1) * 128],
                                in_=ptb[ct][:, 384:512], func=AF.Identity,
                                scale=l1g[:, ct : ct + 1], bias=l1b[:, ct : ct + 1],
                            )

                # ======== phase 2: Q^T, K^T, V (fp8 DoubleRow) ========
                with (
                    pool("wqkv", bufs=1) as wp_,
                    pool("ph2ps", bufs=3, space="PSUM") as ph2ps,
                    pool("ph2pv", bufs=2, space="PSUM") as ph2pv,
                ):
                    wq8 = [wp_.tile([128, 2, C], F8, tag=f"wq{t}", name=f"wq{t}") for t in range(KT)]
                    wk8 = [wp_.tile([128, 2, C], F8, tag=f"wk{t}", name=f"wk{t}") for t in range(KT)]
                    wv8 = [wp_.tile([128, 2, C], F8, tag=f"wv{t}", name=f"wv{t}") for t in range(KT)]
                    for t in range(KT):
                        nc.sync.dma_start(wk8[t][:], d["d_wk"][t, :, :, :])
                    for t in range(KT):
                        nc.sync.dma_start(wv8[t][:], d["d_wv"][t, :, :, :])
                    for t in range(KT):
                        nc.sync.dma_start(wq8[t][:], d["d_wq"][t, :, :, :])
                    # masks on the gpsimd queue, out of the weight path
                    for t in range(TT):
                        nc.gpsimd.dma_start(masks[t][:], d["d_masks"][t, :, :])

                    # kT chunk nn and V tiles 4nn..4nn+3 depend only on
                    # token-group nn of xnT -> overlap with phase 1 tail
                    for nn in range(4):
                        for f in range(CT):
                            ps = ph2ps.tile([128, 512], F32, tag="pqk", name="pk")
                            for kt in range(KT):
                                nc.tensor.matmul(
                                    ps[:], wk8[kt][:, :, f * 128 : (f + 1) * 128],
                                    xnT8[kt][:, :, nn * 512 : (nn + 1) * 512],
                                    start=(kt == 0), stop=(kt == KT - 1),
                                    perf_mode=DR,
                                )
                            nc.vector.tensor_scalar(
                                out=kT[f][:, nn * 512 : (nn + 1) * 512],
                                in0=ps[:], scalar1=bk[:, f : f + 1],
                                scalar2=None, op0=ALU.add,
                            )
                        for tt in range(nn * 4, nn * 4 + 4):
                            pv = ph2pv.tile([128, C], F32, tag="pv", name="pv")
                            for lo, hi in ((0, 512), (512, 768)):
                                for kt in range(KT):
                                    nc.tensor.matmul(
                                        pv[:, lo:hi],
                                        xnT8[kt][:, :, tt * 128 : (tt + 1) * 128],
                                        wv8[kt][:, :, lo:hi],
                                        start=(kt == 0), stop=(kt == KT - 1),
                                        perf_mode=DR,
                                    )
                            vt = V[tt][:].rearrange("p (h e) -> p h e", e=HD + 1)
                            # ones column = WS so the fp8 weight scale cancels
                            # between softmax numerator and denominator
                            nc.vector.memset(vt[:, :, HD : HD + 1], WS)
                            pvh = pv[:].rearrange("p (h e) -> p h e", e=HD)
                            nc.vector.tensor_tensor(
                                out=vt[:, :, 0:HD], in0=pvh[:],
                                in1=bv_bc[:].rearrange("p (h e) -> p h e", e=HD),
                                op=ALU.add,
                            )
                    # qT[f] [128, 512] = (Wq[:, f].T @ xnTq) + bq
                    for f in range(CT):
                        ps = ph2ps.tile([128, NQ], F32, tag="pqk", name="pq")
                        for kt in range(KT):
                            nc.tensor.matmul(
                                ps[:], wq8[kt][:, :, f * 128 : (f + 1) * 128],
                                xnTq8[kt][:], start=(kt == 0), stop=(kt == KT - 1),
                                perf_mode=DR,
                            )
                        nc.vector.tensor_scalar(
                            out=qT[f][:], in0=ps[:], scalar1=bq[:, f : f + 1],
                            scalar2=None, op0=ALU.add,
                        )

            # ======== phase 3: attention (bf16, 4 heads in flight) ========
            # exp is batched over head PAIRS (one ACT op per pair) since ACT
            # runs 1x with a 352-cycle fixed cost per instruction.
            with (
                pool("ph3", bufs=8) as ph3s,
                pool("ph3ps", bufs=2, space="PSUM") as ph3ps,
                pool("ph3pa", bufs=1, space="PSUM") as ph3pa,
            ):
                for hg in range(H // 4):
                    hs = [hg * 4 + i for i in range(4)]
                    pavs = {
                        h: ph3pa.tile(
                            [128, NQ], F32, tag=f"pav{h % 4}", name=f"pav{h % 4}"
                        )
                        for h in hs
                    }
                    for kp in range(TT):
                        cs = 128 * (kp // 4)
                        psbs = {}
                        for pi in range(2):
                            hA, hB = hs[2 * pi], hs[2 * pi + 1]
                            ps2 = ph3ps.tile([128, 2, NQ], F32, tag="ps2", name="ps2")
                            for r, h in ((0, hA), (1, hB)):
                                ro = (h % 2) * 64
                                nc.tensor.matmul(
                                    ps2[:, r, cs:NQ],
                                    kT[h // 2][ro : ro + 64, kp * 128 : (kp + 1) * 128],
                                    qT[h // 2][ro : ro + 64, cs:NQ],
                                )
                            p_sb = ph3s.tile([128, 2, NQ], BF, tag="p_sb", name="p_sb")
                            # q,k both carry the 32x fp8 weight scale
                            nc.scalar.activation(
                                out=p_sb[:, :, cs:NQ], in_=ps2[:, :, cs:NQ],
                                func=AF.Exp, scale=0.125 / (WS * WS),
                            )
                            # only the first in-suffix 128-col block is ever
                            # not all-ones (across every core layout)
                            for r, h in ((0, hA), (1, hB)):
                                nc.vector.tensor_mul(
                                    p_sb[:, r, cs : cs + 128],
                                    p_sb[:, r, cs : cs + 128], masks[kp][:],
                                )
                                psbs[h] = (p_sb, r)
                        for h in hs:
                            p_sb, r = psbs[h]
                            nc.tensor.matmul(
                                pavs[h][0 : HD + 1, cs:NQ],
                                V[kp][:, h * (HD + 1) : (h + 1) * (HD + 1)],
                                p_sb[:, r, cs:NQ],
                                start=(kp == 0), stop=(kp == TT - 1),
                                skip_group_check=True,
                            )
                    for h in hs:
                        ro = (h % 2) * 64
                        pav = pavs[h]
                        den = ph3s.tile([1, NQ], F32, tag="den", name="den")
                        nc.vector.tensor_copy(den[:], pav[HD : HD + 1, :])
                        rbp = ph3ps.tile([128, 2, NQ], F32, tag="ps2", name="rbp")
                        nc.tensor.matmul(rbp[0:64, 0, :], ones64[:], den[:])
                        rb = ph3s.tile([64, NQ], F32, tag="rb", name="rb")
                        nc.vector.reciprocal(out=rb[:], in_=rbp[0:64, 0, :])
                        nc.vector.tensor_tensor(
                            out=yT[h // 2][ro : ro + 64, :],
                            in0=pav[0:HD, :], in1=rb[:], op=ALU.mult,
                        )

        # ======== phase 4: proj + residual + LN2 ========
        with pool("mlp_sb", bufs=1) as mlpp:
            x2 = [mlpp.tile([128, C], F32, tag=f"x2{j}", name=f"x2{j}") for j in range(4)]
            xn2T = [mlpp.tile([128, NQ], BF, tag=f"xn2T{t}", name=f"xn2T{t}") for t in range(CT)]
            hT = [mlpp.tile([128, NQ], BF, tag=f"hT{t}", name=f"hT{t}") for t in range(FT)]
            with (
                pool("mlp1", bufs=1) as m1p,
                pool("mlp1s", bufs=3) as m1s,
            ):
                wp = [m1p.tile([128, C], BF, tag=f"wp{t}", name=f"wp{t}") for t in range(CT)]
                wfc = [m1p.tile([128, DFF], BF, tag=f"wfc{t}", name=f"wfc{t}") for t in range(CT)]
                bfc_bc = m1p.tile([128, DFF], F32, tag="bfcbc", name="bfcbc")
                hh = [m1p.tile([128, DFF], BF, tag=f"hh{j}", name=f"hh{j}") for j in range(4)]
                for t in range(CT):
                    nc.sync.dma_start(wp[t][:], d["d_wp"][t * 128 : (t + 1) * 128, :])
                for t in range(CT):
                    nc.sync.dma_start(wfc[t][:], d["d_wfc"][t * 128 : (t + 1) * 128, :])
                nc.sync.dma_start(bfc_bc[:], d["d_bfc"][:])
                for j in range(4):
                    nc.scalar.dma_start(x_own[j][:], d["d_xob"][j * 128 : (j + 1) * 128, :])

                with (
                    pool("ph4p", bufs=2, space="PSUM") as ph4p,
                    pool("ph4t", bufs=4, space="PSUM") as ph4t,
                ):
                    for qt in range(4):
                        pp = ph4p.tile([128, C], F32, tag="pp", name="pp")
                        for lo, hi in ((0, 512), (512, 768)):
                            for ct in range(CT):
                                nc.tensor.matmul(
                                    pp[:, lo:hi],
                                    yT[ct][:, qt * 128 : (qt + 1) * 128],
                                    wp[ct][:, lo:hi],
                                    start=(ct == 0), stop=(ct == CT - 1),
                                )
                        nc.vector.tensor_add(x2[qt][:], pp[:], x_own[qt][:])
                        mu, rstd = _ln_tile(nc, small, x2[qt][:], eps)
                        xn2 = m1s.tile([128, C], BF, tag="xn2", name="xn2")
                        nc.vector.tensor_scalar(
                            out=xn2[:], in0=x2[qt][:], scalar1=mu, scalar2=rstd[:],
                            op0=ALU.subtract, op1=ALU.mult,
                        )
                        for ct in range(CT):
                            pt = ph4t.tile([128, 128], BF, tag="pt4", name="pt4")
                            nc.tensor.transpose(
                                pt[:], xn2[:, ct * 128 : (ct + 1) * 128], ident[:]
                            )
                            nc.scalar.activation(
                                out=xn2T[ct][:, qt * 128 : (qt + 1) * 128],
                                in_=pt[:], func=AF.Identity,
                                scale=l2g[:, ct : ct + 1], bias=l2b[:, ct : ct + 1],
                            )

                # ======== phase 5: fc + gelu ========
                with pool("ph5p", bufs=2, space="PSUM") as ph5p:
                    for qt in range(4):
                        for nn in range(6):
                            sl = slice(nn * 512, (nn + 1) * 512)
                            ph_ = ph5p.tile([128, 512], F32, tag="ph5", name="ph5")
                            for ct in range(CT):
                                nc.tensor.matmul(
                                    ph_[:],
                                    xn2T[ct][:, qt * 128 : (qt + 1) * 128],
                                    wfc[ct][:, sl],
                                    start=(ct == 0), stop=(ct == CT - 1),
                                )
                            tmp = m1s.tile([128, 512], BF, tag="pregelu", name="pregelu")
                            nc.vector.tensor_add(tmp[:], ph_[:], bfc_bc[:, sl])
                            nc.scalar.activation(
                                out=hh[qt][:, sl], in_=tmp[:], func=AF.Gelu_apprx_tanh,
                            )

                # ======== phase 6: transpose h ========
                with pool("ph6t", bufs=2, space="PSUM") as ph6t:
                    for fc in range(FT):
                        ptb = ph6t.tile([128, 512], BF, tag="pt6", name="pt6")
                        for qt in range(4):
                            nc.tensor.transpose(
                                ptb[:, qt * 128 : (qt + 1) * 128],
                                hh[qt][:, fc * 128 : (fc + 1) * 128], ident[:],
                            )
                        nc.scalar.copy(hT[fc][:], ptb[:])

            # ======== phase 7: fc2 + residual + out ========
            with (
                pool("mlp2", bufs=1) as m2p,
                pool("mlp2s", bufs=3) as m2s,
                pool("ph7p", bufs=2, space="PSUM") as ph7p,
            ):
                wfc2 = [m2p.tile([128, C], BF, tag=f"wfc2{t}", name=f"wfc2{t}") for t in range(FT)]
                for t in range(FT):
                    nc.sync.dma_start(wfc2[t][:], d["d_wfc2"][t * 128 : (t + 1) * 128, :])
                for qt in range(4):
                    po = ph7p.tile([128, C], F32, tag="po", name="po")
                    for lo, hi in ((0, 512), (512, 768)):
                        for kt in range(FT):
                            nc.tensor.matmul(
                                po[:, lo:hi],
                                hT[kt][:, qt * 128 : (qt + 1) * 128],
                                wfc2[kt][:, lo:hi],
                                start=(kt == 0), stop=(kt == FT - 1),
                            )
                    t1 = m2s.tile([128, C], F32, tag="t1", name="t1")
                    nc.vector.tensor_add(t1[:], po[:], bfc2_bc[:])
                    ot = m2s.tile([128, C], F32, tag="ot", name="ot")
                    nc.vector.tensor_add(ot[:], t1[:], x2[qt][:])
                    nc.sync.dma_start(
                        d["d_out"][qt * 128 : (qt + 1) * 128, :], ot[:]
                    )


# ---------------------------------------------------------------------------
# Host-side wrapper
# ---------------------------------------------------------------------------
_PROGRAM = None


def _get_program():
    global _PROGRAM
    if _PROGRAM is None:
        _PROGRAM = build_program()
    return _PROGRAM


def make_in_maps(x, ln1_g, ln1_b, W_attn, b_attn, W_proj, b_proj,
                 ln2_g, ln2_b, W_fc, b_fc, W_fc2, b_fc2):
    x = np.asarray(x, np.float32)
    shared = {
        "wq": pack_dr(W_attn[:, 0:C]),
        "wk": pack_dr(W_attn[:, C : 2 * C]),
        "wv": pack_dr(W_attn[:, 2 * C : 3 * C]),
        "wp": np.asarray(W_proj, BF16),
        "wfc": np.asarray(W_fc, BF16),
        "wfc2": np.asarray(W_fc2, BF16),
        # q/k/v biases ride the 32x weight scale
        "bq2d": np.ascontiguousarray(
            np.asarray(b_attn[0:C], np.float32).reshape(CT, 128).T * WS),
        "bk2d": np.ascontiguousarray(
            np.asarray(b_attn[C : 2 * C], np.float32).reshape(CT, 128).T * WS),
        "bv_bc": np.broadcast_to(
            np.asarray(b_attn[2 * C : 3 * C], np.float32) * WS, (128, C)).copy(),
        "bfc_bc": np.broadcast_to(
            np.asarray(b_fc, np.float32), (128, DFF)).copy(),
        "bfc2_bc": np.broadcast_to(
            np.asarray(b_fc2, np.float32), (128, C)).copy(),
        "ln1g2d": np.ascontiguousarray(
            np.asarray(ln1_g, np.float32).reshape(CT, 128).T),
        "ln1b2d": np.ascontiguousarray(
            np.asarray(ln1_b, np.float32).reshape(CT, 128).T),
        "ln2g2d": np.ascontiguousarray(
            np.asarray(ln2_g, np.float32).reshape(CT, 128).T),
        "ln2b2d": np.ascontiguousarray(
            np.asarray(ln2_b, np.float32).reshape(CT, 128).T),
        "ident": np.eye(128, dtype=BF16),
    }
    bp = np.asarray(b_proj, np.float32)
    in_maps, layouts = [], []
    for core in range(8):
        b, g = core // 4, core % 4
        qtiles, perm = core_layout(g)
        idx = np.concatenate([np.arange(t * 128, (t + 1) * 128) for t in perm])
        own = np.concatenate([np.arange(t * 128, (t + 1) * 128) for t in qtiles])
        m = dict(shared)
        m["x_perm"] = np.ascontiguousarray(x[b][idx])
        m["x_own_b"] = np.ascontiguousarray(x[b][own] + bp)
        m["masks"] = core_masks(qtiles, perm)
        in_maps.append(m)
        layouts.append((b, own))
    return in_maps, layouts


def unshard(results, layouts):
    out = np.empty((B, T, C), np.float32)
    for r, (b, own) in zip(results, layouts):
        out[b][own] = r["out"]
    return out


def kernel(**inputs):
    from concourse.bass_utils import run_bass_kernel_spmd

    nc = _get_program()
    in_maps, layouts = make_in_maps(**inputs)
    res = run_bass_kernel_spmd(nc, in_maps, core_ids=list(range(8)))
    return unshard(res.results, layouts)


# revision 18
# speedup vs baseline: 1.3522x; 1.0082x over previous
"""GPT-2 block (B=2, T=2048, C=768, H=12) on 8 Trainium2 NeuronCores.

Sharding: data-parallel over batch (2) x 4-way query-tile split per batch.
Each core computes K/V for its full batch (avoids on-chip collectives,
whose latency floor exceeds the redundant compute) and runs attention +
MLP for 4 of the 16 query tiles, interleaved {g, 7-g, 8+g, 15-g} so the
causal-attention work is identical across cores.

The SPMD program is uniform across cores: per-core differences are pushed
into the data via a k-tile permutation of the sequence (each core's query
tiles sit at fixed positions {3,7,11,15}; every tile's causal prefix is
placed before it) plus per-core causal masks.

Layouts: activations enter matmuls feature-major (xnT [C,T]) so QKV needs
no transposes; attention scores are computed transposed (S^T [k,q]) so
exp(S^T) is directly the stationary operand of the A*V matmul, and a ones
column appended to V produces the softmax denominator in the same matmul.

Precision: weights and matmul activations are fp8 e4m3 with DoubleRow
matmuls (2 K-rows/cycle, K-tiles of 256). Weights are pre-scaled by 32 on
the host to stay in the fp8 normal range; the scale folds into the exp()
argument for attention (q,k both 32x -> scale/1024), into the V ones
column (=32 so softmax numerator/denominator cancel), and into one cheap
descale per MLP/proj output. Attention S/AV matmuls stay bf16.
"""

import sys

sys.path.insert(0, "/opt/trn_rl_repo")

import numpy as np
import ml_dtypes

import bass_rust
import concourse.bass as bass
import concourse.bacc as bacc
import concourse.tile as tile
from concourse import mybir
from concourse.vector_clock import ScopedClock

BF16 = ml_dtypes.bfloat16
F32 = mybir.dt.float32
BF = mybir.dt.bfloat16
F8 = mybir.dt.float8e4
NP_F8 = mybir.dt.np(F8)

B, T, C, H = 2, 2048, 768, 12
HD = C // H  # 64
DFF = 4 * C  # 3072
TT = T // 128  # 16 token tiles
CT = C // 128  # 6 feature tiles
KT = C // 256  # 3 DoubleRow k-tiles over C
KT2 = DFF // 256  # 12 DoubleRow k-tiles over DFF
FT = DFF // 128  # 24
QPOS = (3, 7, 11, 15)  # fixed positions of this core's query tiles
NQ = 512  # queries per core
WS = 32.0  # fp8 weight pre-scale
AF = mybir.ActivationFunctionType
ALU = mybir.AluOpType
DR = mybir.MatmulPerfMode.DoubleRow

# ---------------------------------------------------------------------------
# Tile exit-drain fix: the final SP drain carries one wait per live logical
# processor, but TRN2 ISA instructions hold at most 1 embedded sync wait in
# this toolchain. Split the waits across a chain of SP drains.
# ---------------------------------------------------------------------------
_MAX_WAITS = 1


def _drain_and_barrier(self, tick_clock, wait_clock):
    drain_inst = self.nc.sync.drain()
    wait_clock.add_sem_waits(
        drain_inst.ins, ScopedClock({None: tick_clock.global_clock})
    )
    si = drain_inst.ins.sync_info
    if si is not None and len(si.on_wait) > _MAX_WAITS:
        waits = list(si.on_wait)
        drain_inst.ins.sync_info = bass_rust.SyncInfo(
            on_wait=waits[:_MAX_WAITS], on_update=list(si.on_update)
        )
        rest = waits[_MAX_WAITS:]
        for i in range(0, len(rest), _MAX_WAITS):
            extra = self.nc.sync.drain()
            extra.ins.sync_info = bass_rust.SyncInfo(
                on_wait=rest[i : i + _MAX_WAITS], on_update=[]
            )
    self.nc.all_engine_barrier()
    assert self.sems is not None
    popped = self.nc._tile_sem_poison_stack.pop()
    assert popped is self._sem_poison
    self.nc.clear_and_free_semaphores(list(self.sems.allocated().values()))
    self.nc.all_engine_barrier()


tile.TileContext._drain_and_barrier = _drain_and_barrier


# ---------------------------------------------------------------------------
# Per-core sharding layout (host side)
# ---------------------------------------------------------------------------
def core_layout(g):
    """For group index g (0..3): (qtiles sorted, perm) with the core's query
    tiles at positions QPOS and every tile's causal prefix placed before it."""
    qtiles = sorted([g, 7 - g, 8 + g, 15 - g])
    posmap = dict(zip(QPOS, qtiles))
    rest = iter([t for t in range(TT) if t not in qtiles])
    perm = [posmap[p] if p in posmap else next(rest) for p in range(TT)]
    # causal validity: tiles <= qtiles[j] all sit at positions <= QPOS[j]
    for j, a in enumerate(qtiles):
        assert set(range(a + 1)) <= set(perm[: QPOS[j] + 1]), (g, j, perm)
    return qtiles, perm


def core_masks(qtiles, perm):
    """masks[kp] = causal mask of k-position kp against query tile j=kp//4
    (the first in-suffix block - across all core layouts the only block
    that is ever not all-ones)."""
    masks = np.zeros((TT, 128, 128), dtype=BF16)
    for kp in range(TT):
        tk = perm[kp] * 128 + np.arange(128)[:, None]
        a = qtiles[kp // 4]
        tq = a * 128 + np.arange(128)[None, :]
        masks[kp] = (tk <= tq).astype(BF16)
    return masks


def pack_dr(W):
    """[K, N] fp32 -> DoubleRow-paired fp8 [K/256, 128, 2, N], pre-scaled.
    Logical k = 256*kt + 128*r + p."""
    K, N = W.shape
    Wp = (np.asarray(W, np.float32) * WS).reshape(K // 256, 2, 128, N)
    return np.ascontiguousarray(Wp.transpose(0, 2, 1, 3)).astype(NP_F8)


# ---------------------------------------------------------------------------
# The Bass program (identical for all 8 cores)
# ---------------------------------------------------------------------------
def build_program():
    nc = bacc.Bacc("TRN2")

    d_x = nc.dram_tensor("x_perm", [T, C], F32, kind="ExternalInput")
    d_xob = nc.dram_tensor("x_own_b", [NQ, C], F32, kind="ExternalInput")
    d_masks = nc.dram_tensor("masks", [TT, 128, 128], BF, kind="ExternalInput")
    d_wq = nc.dram_tensor("wq", [KT, 128, 2, C], F8, kind="ExternalInput")
    d_wk = nc.dram_tensor("wk", [KT, 128, 2, C], F8, kind="ExternalInput")
    d_wv = nc.dram_tensor("wv", [KT, 128, 2, C], F8, kind="ExternalInput")
    d_wp = nc.dram_tensor("wp", [C, C], BF, kind="ExternalInput")
    d_wfc = nc.dram_tensor("wfc", [C, DFF], BF, kind="ExternalInput")
    d_wfc2 = nc.dram_tensor("wfc2", [DFF, C], BF, kind="ExternalInput")
    d_bq = nc.dram_tensor("bq2d", [128, CT], F32, kind="ExternalInput")
    d_bk = nc.dram_tensor("bk2d", [128, CT], F32, kind="ExternalInput")
    d_bv = nc.dram_tensor("bv_bc", [128, C], F32, kind="ExternalInput")
    d_bfc = nc.dram_tensor("bfc_bc", [128, DFF], F32, kind="ExternalInput")
    d_bfc2 = nc.dram_tensor("bfc2_bc", [128, C], F32, kind="ExternalInput")
    d_l1g = nc.dram_tensor("ln1g2d", [128, CT], F32, kind="ExternalInput")
    d_l1b = nc.dram_tensor("ln1b2d", [128, CT], F32, kind="ExternalInput")
    d_l2g = nc.dram_tensor("ln2g2d", [128, CT], F32, kind="ExternalInput")
    d_l2b = nc.dram_tensor("ln2b2d", [128, CT], F32, kind="ExternalInput")
    d_ident = nc.dram_tensor("ident", [128, 128], BF, kind="ExternalInput")
    d_out = nc.dram_tensor("out", [NQ, C], F32, kind="ExternalOutput")

    with tile.TileContext(nc) as tc:
        _body(nc, tc, locals())
    nc.compile()
    return nc


def _ln_tile(nc, pool, x_ap, eps):
    """LayerNorm stats for one [128, C] fp32 tile -> (mu, rstd) [128,1]."""
    stats = pool.tile([128, 3, 6], F32, tag="bnstats", name="bnstats")
    xg = x_ap.rearrange("p (a b) -> p a b", b=256)
    for a in range(3):
        nc.vector.bn_stats(out=stats[:, a, :], in_=xg[:, a, :])
    mv = pool.tile([128, 2], F32, tag="bnaggr", name="bnaggr")
    nc.vector.bn_aggr(out=mv[:], in_=stats[:])
    sd = pool.tile([128, 1], F32, tag="sd", name="sd")
    nc.scalar.activation(out=sd[:], in_=mv[:, 1:2], func=AF.Sqrt, bias=eps[:])
    rstd = pool.tile([128, 1], F32, tag="rstd", name="rstd")
    nc.vector.reciprocal(out=rstd[:], in_=sd[:])
    return mv[:, 0:1], rstd


def _body(nc, tc, d):
    def pool(name, **kw):
        return tc.tile_pool(name=name, **kw)

    with (
        pool("const", bufs=1) as constp,
        pool("persist", bufs=1) as pers,
        pool("small", bufs=6) as small,
    ):
        # ---- constants (tiny DMAs first) --------------------------------
        ident = constp.tile([128, 128], BF)
        nc.sync.dma_start(ident[:], d["d_ident"][:])
        eps = constp.tile([128, 1], F32)
        nc.vector.memset(eps[:], 1e-5)
        l1g = constp.tile([128, CT], F32)
        nc.sync.dma_start(l1g[:], d["d_l1g"][:])
        l1b = constp.tile([128, CT], F32)
        nc.sync.dma_start(l1b[:], d["d_l1b"][:])
        l2g = constp.tile([128, CT], F32)
        nc.sync.dma_start(l2g[:], d["d_l2g"][:])
        l2b = constp.tile([128, CT], F32)
        nc.sync.dma_start(l2b[:], d["d_l2b"][:])
        bq = constp.tile([128, CT], F32)
        nc.sync.dma_start(bq[:], d["d_bq"][:])
        bk = constp.tile([128, CT], F32)
        nc.sync.dma_start(bk[:], d["d_bk"][:])
        bv_bc = constp.tile([128, C], F32)
        nc.sync.dma_start(bv_bc[:], d["d_bv"][:])
        bfc2_bc = constp.tile([128, C], F32)
        nc.sync.dma_start(bfc2_bc[:], d["d_bfc2"][:])
        ones64 = constp.tile([1, 64], F32)
        nc.vector.memset(ones64[:], 1.0)

        # ---- persistent activations -----------------------------------
        x_own = [pers.tile([128, C], F32, tag=f"xo{j}", name=f"xo{j}") for j in range(4)]
        qT = [pers.tile([128, NQ], BF, tag=f"qT{t}", name=f"qT{t}") for t in range(CT)]
        yT = [pers.tile([128, NQ], BF, tag=f"yT{t}", name=f"yT{t}") for t in range(CT)]

        with pool("attn_sb", bufs=1) as attnp:
            kT = [attnp.tile([128, T], BF, tag=f"kT{t}", name=f"kT{t}") for t in range(CT)]
            V = [attnp.tile([128, H * 2 * HD], BF, tag=f"V{t}", name=f"V{t}") for t in range(TT)]
            masks = [attnp.tile([128, 128], BF, tag=f"m{t}", name=f"m{t}") for t in range(TT)]

            # ======== phase 1: LN1 + transpose to feature-major fp8 ========
            with (
                pool("ph1", bufs=1) as ph1p,
                pool("ph1s", bufs=4) as ph1s,
                pool("wqkv", bufs=1) as wp_,
            ):
                xnT8 = [ph1p.tile([128, 2, T], F8, tag=f"xnT{t}", name=f"xnT{t}") for t in range(KT)]
                xnTq8 = [ph1p.tile([128, 2, NQ], F8, tag=f"xnTq{t}", name=f"xnTq{t}") for t in range(KT)]
                wq8 = [wp_.tile([128, 2, C], F8, tag=f"wq{t}", name=f"wq{t}") for t in range(KT)]
                wk8 = [wp_.tile([128, 2, C], F8, tag=f"wk{t}", name=f"wk{t}") for t in range(KT)]
                wv8 = [wp_.tile([128, 2, C], F8, tag=f"wv{t}", name=f"wv{t}") for t in range(KT)]
                for t in range(KT):
                    nc.sync.dma_start(wk8[t][:], d["d_wk"][t, :, :, :])
                for t in range(KT):
                    nc.sync.dma_start(wv8[t][:], d["d_wv"][t, :, :, :])
                for t in range(KT):
                    nc.sync.dma_start(wq8[t][:], d["d_wq"][t, :, :, :])
                for t in range(TT):
                    nc.gpsimd.dma_start(masks[t][:], d["d_masks"][t, :, :])
                with (
                    pool("ph1t", bufs=1, space="PSUM") as ph1t,
                    pool("ph2k", bufs=2, space="PSUM") as ph2k,
                ):
                    for ttg in range(4):
                        ptb = [
                            ph1t.tile([128, 512], BF, tag=f"ptb{t}", name=f"ptb{t}")
                            for t in range(CT)
                        ]
                        for ti in range(4):
                            tt = ttg * 4 + ti
                            xt = ph1s.tile([128, C], F32, tag="xt", name="xt")
                            nc.scalar.dma_start(xt[:], d["d_x"][tt * 128 : (tt + 1) * 128, :])
                            mu, rstd = _ln_tile(nc, small, xt[:], eps)
                            xn = ph1s.tile([128, C], BF, tag="xn", name="xn")
                            nc.vector.tensor_scalar(
                                out=xn[:], in0=xt[:], scalar1=mu, scalar2=rstd[:],
                                op0=ALU.subtract, op1=ALU.mult,
                            )
                            for ct in range(CT):
                                nc.tensor.transpose(
                                    ptb[ct][:, ti * 128 : (ti + 1) * 128],
                                    xn[:, ct * 128 : (ct + 1) * 128], ident[:],
                                )
                        for ct in range(CT):
                            kt, r = ct // 2, ct % 2
                            nc.scalar.activation(
                                out=xnT8[kt][:, r, ttg * 512 : (ttg + 1) * 512],
                                in_=ptb[ct][:], func=AF.Identity,
                                scale=l1g[:, ct : ct + 1], bias=l1b[:, ct : ct + 1],
                            )
                            # own q-tile of this group sits at ti == 3
                            nc.scalar.activation(
                                out=xnTq8[kt][:, r, ttg * 128 : (ttg + 1) * 128],
                                in_=ptb[ct][:, 384:512], func=AF.Identity,
                                scale=l1g[:, ct : ct + 1], bias=l1b[:, ct : ct + 1],
                            )
                        # kT chunk ttg depends only on this token-group
                        for f in range(CT):
                            ps = ph2k.tile([128, 512], F32, tag="pqk", name="pk")
                            for kt in range(KT):
                                nc.tensor.matmul(
                                    ps[:], wk8[kt][:, :, f * 128 : (f + 1) * 128],
                                    xnT8[kt][:, :, ttg * 512 : (ttg + 1) * 512],
                                    start=(kt == 0), stop=(kt == KT - 1),
                                    perf_mode=DR,
                                )
                            nc.vector.tensor_scalar(
                                out=kT[f][:, ttg * 512 : (ttg + 1) * 512],
                                in0=ps[:], scalar1=bk[:, f : f + 1],
                                scalar2=None, op0=ALU.add,
                            )

                # ======== phase 2: Q^T, V (fp8 DoubleRow) ========
                with (
                    pool("ph2ps", bufs=3, space="PSUM") as ph2ps,
                    pool("ph2pv", bufs=2, space="PSUM") as ph2pv,
                ):
                    for nn in range(4):
                        for tt in range(nn * 4, nn * 4 + 4):
                            pv = ph2pv.tile([128, C], F32, tag="pv", name="pv")
                            for lo, hi in ((0, 512), (512, 768)):
                                for kt in range(KT):
                                    nc.tensor.matmul(
                                        pv[:, lo:hi],
                                        xnT8[kt][:, :, tt * 128 : (tt + 1) * 128],
                                        wv8[kt][:, :, lo:hi],
                                        start=(kt == 0), stop=(kt == KT - 1),
                                        perf_mode=DR,
                                    )
                            vt = V[tt][:].rearrange("p (h e) -> p h e", e=2 * HD)
                            # 64 replicated "ones" columns per head: the AV
                            # matmul then lands the softmax denominator in
                            # psum rows 64:128 (value WS cancels the fp8
                            # weight scale between numerator and denominator)
                            nc.vector.memset(vt[:, :, HD : 2 * HD], WS)
                            pvh = pv[:].rearrange("p (h e) -> p h e", e=HD)
                            nc.vector.tensor_tensor(
                                out=vt[:, :, 0:HD], in0=pvh[:],
                                in1=bv_bc[:].rearrange("p (h e) -> p h e", e=HD),
                                op=ALU.add,
                            )
                    # qT[f] [128, 512] = (Wq[:, f].T @ xnTq) + bq
                    for f in range(CT):
                        ps = ph2ps.tile([128, NQ], F32, tag="pqk", name="pq")
                        for kt in range(KT):
                            nc.tensor.matmul(
                                ps[:], wq8[kt][:, :, f * 128 : (f + 1) * 128],
                                xnTq8[kt][:], start=(kt == 0), stop=(kt == KT - 1),
                                perf_mode=DR,
                            )
                        nc.vector.tensor_scalar(
                            out=qT[f][:], in0=ps[:], scalar1=bq[:, f : f + 1],
                            scalar2=None, op0=ALU.add,
                        )

            # ======== phase 3: attention (bf16, 4 heads in flight) ========
            # exp is batched over head PAIRS (one ACT op per pair) since ACT
            # runs 1x with a 352-cycle fixed cost per instruction.
            with (
                pool("ph3", bufs=8) as ph3s,
                pool("ph3ps", bufs=2, space="PSUM") as ph3ps,
                pool("ph3pa", bufs=1, space="PSUM") as ph3pa,
            ):
                for hg in range(H // 4):
                    hs = [hg * 4 + i for i in range(4)]
                    pavs = {
                        h: ph3pa.tile(
                            [128, NQ], F32, tag=f"pav{h % 4}", name=f"pav{h % 4}"
                        )
                        for h in hs
                    }
                    for kp in range(TT):
                        cs = 128 * (kp // 4)
                        psbs = {}
                        for pi in range(2):
                            hA, hB = hs[2 * pi], hs[2 * pi + 1]
                            ps2 = ph3ps.tile([128, 2, NQ], F32, tag="ps2", name="ps2")
                            for r, h in ((0, hA), (1, hB)):
                                ro = (h % 2) * 64
                                nc.tensor.matmul(
                                    ps2[:, r, cs:NQ],
                                    kT[h // 2][ro : ro + 64, kp * 128 : (kp + 1) * 128],
                                    qT[h // 2][ro : ro + 64, cs:NQ],
                                )
                            p_sb = ph3s.tile([128, 2, NQ], BF, tag="p_sb", name="p_sb")
                            # q,k both carry the 32x fp8 weight scale
                            nc.scalar.activation(
                                out=p_sb[:, :, cs:NQ], in_=ps2[:, :, cs:NQ],
                                func=AF.Exp, scale=0.125 / (WS * WS),
                            )
                            # only the first in-suffix 128-col block is ever
                            # not all-ones (across every core layout)
                            for r, h in ((0, hA), (1, hB)):
                                nc.vector.tensor_mul(
                                    p_sb[:, r, cs : cs + 128],
                                    p_sb[:, r, cs : cs + 128], masks[kp][:],
                                )
                                psbs[h] = (p_sb, r)
                        for h in hs:
                            p_sb, r = psbs[h]
                            nc.tensor.matmul(
                                pavs[h][:, cs:NQ],
                                V[kp][:, h * 2 * HD : (h + 1) * 2 * HD],
                                p_sb[:, r, cs:NQ],
                                start=(kp == 0), stop=(kp == TT - 1),
                                skip_group_check=True,
                            )
                    for h in hs:
                        ro = (h % 2) * 64
                        pav = pavs[h]
                        den = ph3s.tile([64, NQ], F32, tag="den", name="den")
                        nc.scalar.copy(den[:], pav[HD : 2 * HD, :])
                        rb = ph3s.tile([64, NQ], F32, tag="rb", name="rb")
                        nc.vector.reciprocal(out=rb[:], in_=den[:])
                        nc.vector.tensor_tensor(
                            out=yT[h // 2][ro : ro + 64, :],
                            in0=pav[0:HD, :], in1=rb[:], op=ALU.mult,
                        )

        # ======== phase 4: proj + residual + LN2 ========
        with pool("mlp_sb", bufs=1) as mlpp:
            x2 = [mlpp.tile([128, C], F32, tag=f"x2{j}", name=f"x2{j}") for j in range(4)]
            xn2T = [mlpp.tile([128, NQ], BF, tag=f"xn2T{t}", name=f"xn2T{t}") for t in range(CT)]
            hT = [mlpp.tile([128, NQ], BF, tag=f"hT{t}", name=f"hT{t}") for t in range(FT)]
            with (
                pool("mlp1", bufs=1) as m1p,
                pool("mlp1s", bufs=3) as m1s,
            ):
                wp = [m1p.tile([128, C], BF, tag=f"wp{t}", name=f"wp{t}") for t in range(CT)]
                wfc = [m1p.tile([128, DFF], BF, tag=f"wfc{t}", name=f"wfc{t}") for t in range(CT)]
                bfc_bc = m1p.tile([128, DFF], F32, tag="bfcbc", name="bfcbc")
                hh = [m1p.tile([128, DFF], BF, tag=f"hh{j}", name=f"hh{j}") for j in range(4)]
                for t in range(CT):
                    nc.sync.dma_start(wp[t][:], d["d_wp"][t * 128 : (t + 1) * 128, :])
                for t in range(CT):
                    nc.sync.dma_start(wfc[t][:], d["d_wfc"][t * 128 : (t + 1) * 128, :])
                nc.sync.dma_start(bfc_bc[:], d["d_bfc"][:])
                for j in range(4):
                    nc.scalar.dma_start(x_own[j][:], d["d_xob"][j * 128 : (j + 1) * 128, :])

                with (
                    pool("ph4p", bufs=2, space="PSUM") as ph4p,
                    pool("ph4t", bufs=4, space="PSUM") as ph4t,
                ):
                    for qt in range(4):
                        pp = ph4p.tile([128, C], F32, tag="pp", name="pp")
                        for lo, hi in ((0, 512), (512, 768)):
                            for ct in range(CT):
                                nc.tensor.matmul(
                                    pp[:, lo:hi],
                                    yT[ct][:, qt * 128 : (qt + 1) * 128],
                                    wp[ct][:, lo:hi],
                                    start=(ct == 0), stop=(ct == CT - 1),
                                )
                        nc.vector.tensor_add(x2[qt][:], pp[:], x_own[qt][:])
                        mu, rstd = _ln_tile(nc, small, x2[qt][:], eps)
                        xn2 = m1s.tile([128, C], BF, tag="xn2", name="xn2")
                        nc.vector.tensor_scalar(
                            out=xn2[:], in0=x2[qt][:], scalar1=mu, scalar2=rstd[:],
                            op0=ALU.subtract, op1=ALU.mult,
                        )
                        for ct in range(CT):
                            pt = ph4t.tile([128, 128], BF, tag="pt4", name="pt4")
                            nc.tensor.transpose(
                                pt[:], xn2[:, ct * 128 : (ct + 1) * 128], ident[:]
                            )
                            nc.scalar.activation(
                                out=xn2T[ct][:, qt * 128 : (qt + 1) * 128],
                                in_=pt[:], func=AF.Identity,
                                scale=l2g[:, ct : ct + 1], bias=l2b[:, ct : ct + 1],
                            )

                # ======== phase 5: fc + gelu ========
                with pool("ph5p", bufs=2, space="PSUM") as ph5p:
                    for qt in range(4):
                        for nn in range(6):
                            sl = slice(nn * 512, (nn + 1) * 512)
                            ph_ = ph5p.tile([128, 512], F32, tag="ph5", name="ph5")
                            for ct in range(CT):
                                nc.tensor.matmul(
                                    ph_[:],
                                    xn2T[ct][:, qt * 128 : (qt + 1) * 128],
                                    wfc[ct][:, sl],
                                    start=(ct == 0), stop=(ct == CT - 1),
                                )
                            tmp = m1s.tile([128, 512], BF, tag="pregelu", name="pregelu")
                            nc.vector.tensor_add(tmp[:], ph_[:], bfc_bc[:, sl])
                            nc.scalar.activation(
                                out=hh[qt][:, sl], in_=tmp[:], func=AF.Gelu_apprx_tanh,
                            )

                # ======== phase 6: transpose h ========
                with pool("ph6t", bufs=2, space="PSUM") as ph6t:
                    for fc in range(FT):
                        ptb = ph6t.tile([128, 512], BF, tag="pt6", name="pt6")
                        for qt in range(4):
                            nc.tensor.transpose(
                                ptb[:, qt * 128 : (qt + 1) * 128],
                                hh[qt][:, fc * 128 : (fc + 1) * 128], ident[:],
                            )
                        nc.scalar.copy(hT[fc][:], ptb[:])

            # ======== phase 7: fc2 + residual + out ========
            with (
                pool("mlp2", bufs=1) as m2p,
                pool("mlp2s", bufs=3) as m2s,
                pool("ph7p", bufs=2, space="PSUM") as ph7p,
            ):
                wfc2 = [m2p.tile([128, C], BF, tag=f"wfc2{t}", name=f"wfc2{t}") for t in range(FT)]
                for t in range(FT):
                    nc.sync.dma_start(wfc2[t][:], d["d_wfc2"][t * 128 : (t + 1) * 128, :])
                for qt in range(4):
                    po = ph7p.tile([128, C], F32, tag="po", name="po")
                    for lo, hi in ((0, 512), (512, 768)):
                        for kt in range(FT):
                            nc.tensor.matmul(
                                po[:, lo:hi],
                                hT[kt][:, qt * 128 : (qt + 1) * 128],
                                wfc2[kt][:, lo:hi],
                                start=(kt == 0), stop=(kt == FT - 1),
                            )
                    t1 = m2s.tile([128, C], F32, tag="t1", name="t1")
                    nc.vector.tensor_add(t1[:], po[:], bfc2_bc[:])
                    ot = m2s.tile([128, C], F32, tag="ot", name="ot")
                    nc.vector.tensor_add(ot[:], t1[:], x2[qt][:])
                    nc.sync.dma_start(
                        d["d_out"][qt * 128 : (qt + 1) * 128, :], ot[:]
                    )


# ---------------------------------------------------------------------------
# Host-side wrapper
# ---------------------------------------------------------------------------
_PROGRAM = None


def _get_program():
    global _PROGRAM
    if _PROGRAM is None:
        _PROGRAM = build_program()
    return _PROGRAM


def make_in_maps(x, ln1_g, ln1_b, W_attn, b_attn, W_proj, b_proj,
                 ln2_g, ln2_b, W_fc, b_fc, W_fc2, b_fc2):
    x = np.asarray(x, np.float32)
    shared = {
        "wq": pack_dr(W_attn[:, 0:C]),
        "wk": pack_dr(W_attn[:, C : 2 * C]),
        "wv": pack_dr(W_attn[:, 2 * C : 3 * C]),
        "wp": np.asarray(W_proj, BF16),
        "wfc": np.asarray(W_fc, BF16),
        "wfc2": np.asarray(W_fc2, BF16),
        # q/k/v biases ride the 32x weight scale
        "bq2d": np.ascontiguousarray(
            np.asarray(b_attn[0:C], np.float32).reshape(CT, 128).T * WS),
        "bk2d": np.ascontiguousarray(
            np.asarray(b_attn[C : 2 * C], np.float32).reshape(CT, 128).T * WS),
        "bv_bc": np.broadcast_to(
            np.asarray(b_attn[2 * C : 3 * C], np.float32) * WS, (128, C)).copy(),
        "bfc_bc": np.broadcast_to(
            np.asarray(b_fc, np.float32), (128, DFF)).copy(),
        "bfc2_bc": np.broadcast_to(
            np.asarray(b_fc2, np.float32), (128, C)).copy(),
        "ln1g2d": np.ascontiguousarray(
            np.asarray(ln1_g, np.float32).reshape(CT, 128).T),
        "ln1b2d": np.ascontiguousarray(
            np.asarray(ln1_b, np.float32).reshape(CT, 128).T),
        "ln2g2d": np.ascontiguousarray(
            np.asarray(ln2_g, np.float32).reshape(CT, 128).T),
        "ln2b2d": np.ascontiguousarray(
            np.asarray(ln2_b, np.float32).reshape(CT, 128).T),
        "ident": np.eye(128, dtype=BF16),
    }
    bp = np.asarray(b_proj, np.float32)
    in_maps, layouts = [], []
    for core in range(8):
        b, g = core // 4, core % 4
        qtiles, perm = core_layout(g)
        idx = np.concatenate([np.arange(t * 128, (t + 1) * 128) for t in perm])
        own = np.concatenate([np.arange(t * 128, (t + 1) * 128) for t in qtiles])
        m = dict(shared)
        m["x_perm"] = np.ascontiguousarray(x[b][idx])
        m["x_own_b"] = np.ascontiguousarray(x[b][own] + bp)
        m["masks"] = core_masks(qtiles, perm)
        in_maps.append(m)
        layouts.append((b, own))
    return in_maps, layouts


def unshard(results, layouts):
    out = np.empty((B, T, C), np.float32)
    for r, (b, own) in zip(results, layouts):
        out[b][own] = r["out"]
    return out


def kernel(**inputs):
    from concourse.bass_utils import run_bass_kernel_spmd

    nc = _get_program()
    in_maps, layouts = make_in_maps(**inputs)
    res = run_bass_kernel_spmd(nc, in_maps, core_ids=list(range(8)))
    return unshard(res.results, layouts)


# revision 19
# speedup vs baseline: 1.3739x; 1.0161x over previous
"""GPT-2 block (B=2, T=2048, C=768, H=12) on 8 Trainium2 NeuronCores.

Sharding: data-parallel over batch (2) x 4-way query-tile split per batch.
Each core computes K/V for its full batch (avoids on-chip collectives,
whose latency floor exceeds the redundant compute) and runs attention +
MLP for 4 of the 16 query tiles, interleaved {g, 7-g, 8+g, 15-g} so the
causal-attention work is identical across cores.

The SPMD program is uniform across cores: per-core differences are pushed
into the data via a k-tile permutation of the sequence (each core's query
tiles sit at fixed positions {3,7,11,15}; every tile's causal prefix is
placed before it) plus per-core causal masks.

Layouts: activations enter matmuls feature-major (xnT [C,T]) so QKV needs
no transposes; attention scores are computed transposed (S^T [k,q]) so
exp(S^T) is directly the stationary operand of the A*V matmul, and a ones
column appended to V produces the softmax denominator in the same matmul.

Precision: weights and matmul activations are fp8 e4m3 with DoubleRow
matmuls (2 K-rows/cycle, K-tiles of 256). Weights are pre-scaled by 32 on
the host to stay in the fp8 normal range; the scale folds into the exp()
argument for attention (q,k both 32x -> scale/1024), into the V ones
column (=32 so softmax numerator/denominator cancel), and into one cheap
descale per MLP/proj output. Attention S/AV matmuls stay bf16.
"""

import sys

sys.path.insert(0, "/opt/trn_rl_repo")

import numpy as np
import ml_dtypes

import bass_rust
import concourse.bass as bass
import concourse.bacc as bacc
import concourse.tile as tile
from concourse import mybir
from concourse.vector_clock import ScopedClock

BF16 = ml_dtypes.bfloat16
F32 = mybir.dt.float32
BF = mybir.dt.bfloat16
F8 = mybir.dt.float8e4
NP_F8 = mybir.dt.np(F8)

B, T, C, H = 2, 2048, 768, 12
HD = C // H  # 64
DFF = 4 * C  # 3072
TT = T // 128  # 16 token tiles
CT = C // 128  # 6 feature tiles
KT = C // 256  # 3 DoubleRow k-tiles over C
KT2 = DFF // 256  # 12 DoubleRow k-tiles over DFF
FT = DFF // 128  # 24
QPOS = (3, 7, 11, 15)  # fixed positions of this core's query tiles
NQ = 512  # queries per core
WS = 32.0  # fp8 weight pre-scale
AF = mybir.ActivationFunctionType
ALU = mybir.AluOpType
DR = mybir.MatmulPerfMode.DoubleRow

# ---------------------------------------------------------------------------
# Tile exit-drain fix: the final SP drain carries one wait per live logical
# processor, but TRN2 ISA instructions hold at most 1 embedded sync wait in
# this toolchain. Split the waits across a chain of SP drains.
# ---------------------------------------------------------------------------
_MAX_WAITS = 1


def _drain_and_barrier(self, tick_clock, wait_clock):
    drain_inst = self.nc.sync.drain()
    wait_clock.add_sem_waits(
        drain_inst.ins, ScopedClock({None: tick_clock.global_clock})
    )
    si = drain_inst.ins.sync_info
    if si is not None and len(si.on_wait) > _MAX_WAITS:
        waits = list(si.on_wait)
        drain_inst.ins.sync_info = bass_rust.SyncInfo(
            on_wait=waits[:_MAX_WAITS], on_update=list(si.on_update)
        )
        rest = waits[_MAX_WAITS:]
        for i in range(0, len(rest), _MAX_WAITS):
            extra = self.nc.sync.drain()
            extra.ins.sync_info = bass_rust.SyncInfo(
                on_wait=rest[i : i + _MAX_WAITS], on_update=[]
            )
    self.nc.all_engine_barrier()
    assert self.sems is not None
    popped = self.nc._tile_sem_poison_stack.pop()
    assert popped is self._sem_poison
    self.nc.clear_and_free_semaphores(list(self.sems.allocated().values()))
    self.nc.all_engine_barrier()


tile.TileContext._drain_and_barrier = _drain_and_barrier


# ---------------------------------------------------------------------------
# Per-core sharding layout (host side)
# ---------------------------------------------------------------------------
def core_layout(g):
    """For group index g (0..3): (qtiles sorted, perm) with the core's query
    tiles at positions QPOS and every tile's causal prefix placed before it."""
    qtiles = sorted([g, 7 - g, 8 + g, 15 - g])
    posmap = dict(zip(QPOS, qtiles))
    rest = iter([t for t in range(TT) if t not in qtiles])
    perm = [posmap[p] if p in posmap else next(rest) for p in range(TT)]
    # causal validity: tiles <= qtiles[j] all sit at positions <= QPOS[j]
    for j, a in enumerate(qtiles):
        assert set(range(a + 1)) <= set(perm[: QPOS[j] + 1]), (g, j, perm)
    return qtiles, perm


def core_masks(qtiles, perm):
    """masks[kp] = causal mask of k-position kp against query tile j=kp//4
    (the first in-suffix block - across all core layouts the only block
    that is ever not all-ones)."""
    masks = np.zeros((TT, 128, 128), dtype=BF16)
    for kp in range(TT):
        tk = perm[kp] * 128 + np.arange(128)[:, None]
        a = qtiles[kp // 4]
        tq = a * 128 + np.arange(128)[None, :]
        masks[kp] = (tk <= tq).astype(BF16)
    return masks


def pack_dr(W):
    """[K, N] fp32 -> DoubleRow-paired fp8 [K/256, 128, 2, N], pre-scaled.
    Logical k = 256*kt + 128*r + p."""
    K, N = W.shape
    Wp = (np.asarray(W, np.float32) * WS).reshape(K // 256, 2, 128, N)
    return np.ascontiguousarray(Wp.transpose(0, 2, 1, 3)).astype(NP_F8)


# ---------------------------------------------------------------------------
# The Bass program (identical for all 8 cores)
# ---------------------------------------------------------------------------
def build_program():
    nc = bacc.Bacc("TRN2")

    d_x = nc.dram_tensor("x_perm", [T, C], F32, kind="ExternalInput")
    d_xob = nc.dram_tensor("x_own_b", [NQ, C], F32, kind="ExternalInput")
    d_masks = nc.dram_tensor("masks", [TT, 128, 128], BF, kind="ExternalInput")
    d_wq = nc.dram_tensor("wq", [KT, 128, 2, C], F8, kind="ExternalInput")
    d_wk = nc.dram_tensor("wk", [KT, 128, 2, C], F8, kind="ExternalInput")
    d_wv = nc.dram_tensor("wv", [KT, 128, 2, C], F8, kind="ExternalInput")
    d_wp = nc.dram_tensor("wp", [C, C], BF, kind="ExternalInput")
    d_wfc = nc.dram_tensor("wfc", [C, DFF], BF, kind="ExternalInput")
    d_wfc2 = nc.dram_tensor("wfc2", [DFF, C], BF, kind="ExternalInput")
    d_bq = nc.dram_tensor("bq2d", [128, CT], F32, kind="ExternalInput")
    d_bk = nc.dram_tensor("bk2d", [128, CT], F32, kind="ExternalInput")
    d_bv = nc.dram_tensor("bv_bc", [128, C], F32, kind="ExternalInput")
    d_bfc = nc.dram_tensor("bfc_bc", [128, DFF], F32, kind="ExternalInput")
    d_bfc2 = nc.dram_tensor("bfc2_bc", [128, C], F32, kind="ExternalInput")
    d_l1g = nc.dram_tensor("ln1g2d", [128, CT], F32, kind="ExternalInput")
    d_l1b = nc.dram_tensor("ln1b2d", [128, CT], F32, kind="ExternalInput")
    d_l2g = nc.dram_tensor("ln2g2d", [128, CT], F32, kind="ExternalInput")
    d_l2b = nc.dram_tensor("ln2b2d", [128, CT], F32, kind="ExternalInput")
    d_ident = nc.dram_tensor("ident", [128, 128], BF, kind="ExternalInput")
    d_out = nc.dram_tensor("out", [NQ, C], F32, kind="ExternalOutput")

    with tile.TileContext(nc) as tc:
        _body(nc, tc, locals())
    nc.compile()
    return nc


def _ln_tile(nc, pool, x_ap, eps):
    """LayerNorm stats for one [128, C] fp32 tile -> (mu, rstd) [128,1]."""
    stats = pool.tile([128, 3, 6], F32, tag="bnstats", name="bnstats")
    xg = x_ap.rearrange("p (a b) -> p a b", b=256)
    for a in range(3):
        nc.vector.bn_stats(out=stats[:, a, :], in_=xg[:, a, :])
    mv = pool.tile([128, 2], F32, tag="bnaggr", name="bnaggr")
    nc.vector.bn_aggr(out=mv[:], in_=stats[:])
    sd = pool.tile([128, 1], F32, tag="sd", name="sd")
    nc.scalar.activation(out=sd[:], in_=mv[:, 1:2], func=AF.Sqrt, bias=eps[:])
    rstd = pool.tile([128, 1], F32, tag="rstd", name="rstd")
    nc.vector.reciprocal(out=rstd[:], in_=sd[:])
    return mv[:, 0:1], rstd


def _body(nc, tc, d):
    def pool(name, **kw):
        return tc.tile_pool(name=name, **kw)

    with (
        pool("const", bufs=1) as constp,
        pool("persist", bufs=1) as pers,
        pool("small", bufs=6) as small,
    ):
        # ---- constants (tiny DMAs first) --------------------------------
        ident = constp.tile([128, 128], BF)
        nc.sync.dma_start(ident[:], d["d_ident"][:])
        eps = constp.tile([128, 1], F32)
        nc.vector.memset(eps[:], 1e-5)
        l1g = constp.tile([128, CT], F32)
        nc.sync.dma_start(l1g[:], d["d_l1g"][:])
        l1b = constp.tile([128, CT], F32)
        nc.sync.dma_start(l1b[:], d["d_l1b"][:])
        l2g = constp.tile([128, CT], F32)
        nc.sync.dma_start(l2g[:], d["d_l2g"][:])
        l2b = constp.tile([128, CT], F32)
        nc.sync.dma_start(l2b[:], d["d_l2b"][:])
        bq = constp.tile([128, CT], F32)
        nc.sync.dma_start(bq[:], d["d_bq"][:])
        bk = constp.tile([128, CT], F32)
        nc.sync.dma_start(bk[:], d["d_bk"][:])
        bv_bc = constp.tile([128, C], F32)
        nc.sync.dma_start(bv_bc[:], d["d_bv"][:])
        bfc2_bc = constp.tile([128, C], F32)
        nc.sync.dma_start(bfc2_bc[:], d["d_bfc2"][:])
        ones64 = constp.tile([1, 64], F32)
        nc.vector.memset(ones64[:], 1.0)

        # ---- persistent activations -----------------------------------
        wp = [pers.tile([128, C], BF, tag=f"wp{t}", name=f"wp{t}") for t in range(CT)]
        wfc = [pers.tile([128, DFF], BF, tag=f"wfc{t}", name=f"wfc{t}") for t in range(CT)]
        x_own = [pers.tile([128, C], F32, tag=f"xo{j}", name=f"xo{j}") for j in range(4)]
        qT = [pers.tile([128, NQ], BF, tag=f"qT{t}", name=f"qT{t}") for t in range(CT)]
        yT = [pers.tile([128, NQ], BF, tag=f"yT{t}", name=f"yT{t}") for t in range(CT)]

        with pool("attn_sb", bufs=1) as attnp:
            kT = [attnp.tile([128, T], BF, tag=f"kT{t}", name=f"kT{t}") for t in range(CT)]
            V = [attnp.tile([128, H * 2 * HD], BF, tag=f"V{t}", name=f"V{t}") for t in range(TT)]
            masks = [attnp.tile([128, 128], BF, tag=f"m{t}", name=f"m{t}") for t in range(TT)]

            # ======== phase 1: LN1 + transpose to feature-major fp8 ========
            with (
                pool("ph1", bufs=1) as ph1p,
                pool("ph1s", bufs=4) as ph1s,
                pool("wqkv", bufs=1) as wp_,
            ):
                xnT8 = [ph1p.tile([128, 2, T], F8, tag=f"xnT{t}", name=f"xnT{t}") for t in range(KT)]
                xnTq8 = [ph1p.tile([128, 2, NQ], F8, tag=f"xnTq{t}", name=f"xnTq{t}") for t in range(KT)]
                wq8 = [wp_.tile([128, 2, C], F8, tag=f"wq{t}", name=f"wq{t}") for t in range(KT)]
                wk8 = [wp_.tile([128, 2, C], F8, tag=f"wk{t}", name=f"wk{t}") for t in range(KT)]
                wv8 = [wp_.tile([128, 2, C], F8, tag=f"wv{t}", name=f"wv{t}") for t in range(KT)]
                for t in range(KT):
                    nc.sync.dma_start(wk8[t][:], d["d_wk"][t, :, :, :])
                for t in range(KT):
                    nc.sync.dma_start(wv8[t][:], d["d_wv"][t, :, :, :])
                for t in range(KT):
                    nc.sync.dma_start(wq8[t][:], d["d_wq"][t, :, :, :])
                for t in range(TT):
                    nc.gpsimd.dma_start(masks[t][:], d["d_masks"][t, :, :])
                for t in range(CT):
                    nc.sync.dma_start(wp[t][:], d["d_wp"][t * 128 : (t + 1) * 128, :])
                for t in range(CT):
                    nc.sync.dma_start(wfc[t][:], d["d_wfc"][t * 128 : (t + 1) * 128, :])
                for j in range(4):
                    nc.scalar.dma_start(x_own[j][:], d["d_xob"][j * 128 : (j + 1) * 128, :])
                with (
                    pool("ph1t", bufs=1, space="PSUM") as ph1t,
                    pool("ph2k", bufs=2, space="PSUM") as ph2k,
                ):
                    for ttg in range(4):
                        ptb = [
                            ph1t.tile([128, 512], BF, tag=f"ptb{t}", name=f"ptb{t}")
                            for t in range(CT)
                        ]
                        for ti in range(4):
                            tt = ttg * 4 + ti
                            xt = ph1s.tile([128, C], F32, tag="xt", name="xt")
                            nc.scalar.dma_start(xt[:], d["d_x"][tt * 128 : (tt + 1) * 128, :])
                            mu, rstd = _ln_tile(nc, small, xt[:], eps)
                            xn = ph1s.tile([128, C], BF, tag="xn", name="xn")
                            nc.vector.tensor_scalar(
                                out=xn[:], in0=xt[:], scalar1=mu, scalar2=rstd[:],
                                op0=ALU.subtract, op1=ALU.mult,
                            )
                            for ct in range(CT):
                                nc.tensor.transpose(
                                    ptb[ct][:, ti * 128 : (ti + 1) * 128],
                                    xn[:, ct * 128 : (ct + 1) * 128], ident[:],
                                )
                        for ct in range(CT):
                            kt, r = ct // 2, ct % 2
                            nc.scalar.activation(
                                out=xnT8[kt][:, r, ttg * 512 : (ttg + 1) * 512],
                                in_=ptb[ct][:], func=AF.Identity,
                                scale=l1g[:, ct : ct + 1], bias=l1b[:, ct : ct + 1],
                            )
                            # own q-tile of this group sits at ti == 3
                            nc.scalar.activation(
                                out=xnTq8[kt][:, r, ttg * 128 : (ttg + 1) * 128],
                                in_=ptb[ct][:, 384:512], func=AF.Identity,
                                scale=l1g[:, ct : ct + 1], bias=l1b[:, ct : ct + 1],
                            )
                        # kT chunk ttg depends only on this token-group
                        for f in range(CT):
                            ps = ph2k.tile([128, 512], F32, tag="pqk", name="pk")
                            for kt in range(KT):
                                nc.tensor.matmul(
                                    ps[:], wk8[kt][:, :, f * 128 : (f + 1) * 128],
                                    xnT8[kt][:, :, ttg * 512 : (ttg + 1) * 512],
                                    start=(kt == 0), stop=(kt == KT - 1),
                                    perf_mode=DR,
                                )
                            nc.vector.tensor_scalar(
                                out=kT[f][:, ttg * 512 : (ttg + 1) * 512],
                                in0=ps[:], scalar1=bk[:, f : f + 1],
                                scalar2=None, op0=ALU.add,
                            )

                # ======== phase 2: Q^T, V (fp8 DoubleRow) ========
                with (
                    pool("ph2ps", bufs=3, space="PSUM") as ph2ps,
                    pool("ph2pv", bufs=2, space="PSUM") as ph2pv,
                ):
                    for nn in range(4):
                        for tt in range(nn * 4, nn * 4 + 4):
                            pv = ph2pv.tile([128, C], F32, tag="pv", name="pv")
                            for lo, hi in ((0, 512), (512, 768)):
                                for kt in range(KT):
                                    nc.tensor.matmul(
                                        pv[:, lo:hi],
                                        xnT8[kt][:, :, tt * 128 : (tt + 1) * 128],
                                        wv8[kt][:, :, lo:hi],
                                        start=(kt == 0), stop=(kt == KT - 1),
                                        perf_mode=DR,
                                    )
                            vt = V[tt][:].rearrange("p (h e) -> p h e", e=2 * HD)
                            # 64 replicated "ones" columns per head: the AV
                            # matmul then lands the softmax denominator in
                            # psum rows 64:128 (value WS cancels the fp8
                            # weight scale between numerator and denominator)
                            nc.vector.memset(vt[:, :, HD : 2 * HD], WS)
                            pvh = pv[:].rearrange("p (h e) -> p h e", e=HD)
                            nc.vector.tensor_tensor(
                                out=vt[:, :, 0:HD], in0=pvh[:],
                                in1=bv_bc[:].rearrange("p (h e) -> p h e", e=HD),
                                op=ALU.add,
                            )
                    # qT[f] [128, 512] = (Wq[:, f].T @ xnTq) + bq
                    for f in range(CT):
                        ps = ph2ps.tile([128, NQ], F32, tag="pqk", name="pq")
                        for kt in range(KT):
                            nc.tensor.matmul(
                                ps[:], wq8[kt][:, :, f * 128 : (f + 1) * 128],
                                xnTq8[kt][:], start=(kt == 0), stop=(kt == KT - 1),
                                perf_mode=DR,
                            )
                        nc.vector.tensor_scalar(
                            out=qT[f][:], in0=ps[:], scalar1=bq[:, f : f + 1],
                            scalar2=None, op0=ALU.add,
                        )

            # ======== phase 3: attention (bf16, 4 heads in flight) ========
            # exp is batched over head PAIRS (one ACT op per pair) since ACT
            # runs 1x with a 352-cycle fixed cost per instruction.
            with (
                pool("ph3", bufs=8) as ph3s,
                pool("ph3ps", bufs=2, space="PSUM") as ph3ps,
                pool("ph3pa", bufs=1, space="PSUM") as ph3pa,
            ):
                for hg in range(H // 4):
                    hs = [hg * 4 + i for i in range(4)]
                    pavs = {
                        h: ph3pa.tile(
                            [128, NQ], F32, tag=f"pav{h % 4}", name=f"pav{h % 4}"
                        )
                        for h in hs
                    }
                    for kp in range(TT):
                        cs = 128 * (kp // 4)
                        psbs = {}
                        for pi in range(2):
                            hA, hB = hs[2 * pi], hs[2 * pi + 1]
                            ps2 = ph3ps.tile([128, 2, NQ], F32, tag="ps2", name="ps2")
                            for r, h in ((0, hA), (1, hB)):
                                ro = (h % 2) * 64
                                nc.tensor.matmul(
                                    ps2[:, r, cs:NQ],
                                    kT[h // 2][ro : ro + 64, kp * 128 : (kp + 1) * 128],
                                    qT[h // 2][ro : ro + 64, cs:NQ],
                                )
                            p_sb = ph3s.tile([128, 2, NQ], BF, tag="p_sb", name="p_sb")
                            # q,k both carry the 32x fp8 weight scale
                            nc.scalar.activation(
                                out=p_sb[:, :, cs:NQ], in_=ps2[:, :, cs:NQ],
                                func=AF.Exp, scale=0.125 / (WS * WS),
                            )
                            # only the first in-suffix 128-col block is ever
                            # not all-ones (across every core layout)
                            for r, h in ((0, hA), (1, hB)):
                                nc.vector.tensor_mul(
                                    p_sb[:, r, cs : cs + 128],
                                    p_sb[:, r, cs : cs + 128], masks[kp][:],
                                )
                                psbs[h] = (p_sb, r)
                        for h in hs:
                            p_sb, r = psbs[h]
                            nc.tensor.matmul(
                                pavs[h][:, cs:NQ],
                                V[kp][:, h * 2 * HD : (h + 1) * 2 * HD],
                                p_sb[:, r, cs:NQ],
                                start=(kp == 0), stop=(kp == TT - 1),
                                skip_group_check=True,
                            )
                    for h in hs:
                        ro = (h % 2) * 64
                        pav = pavs[h]
                        den = ph3s.tile([64, NQ], F32, tag="den", name="den")
                        nc.vector.tensor_copy(den[:], pav[HD : 2 * HD, :])
                        rb = ph3s.tile([64, NQ], F32, tag="rb", name="rb")
                        nc.vector.reciprocal(out=rb[:], in_=den[:])
                        nc.vector.tensor_tensor(
                            out=yT[h // 2][ro : ro + 64, :],
                            in0=pav[0:HD, :], in1=rb[:], op=ALU.mult,
                        )

        # ======== phase 4: proj + residual + LN2 ========
        with pool("mlp_sb", bufs=1) as mlpp:
            x2 = [mlpp.tile([128, C], F32, tag=f"x2{j}", name=f"x2{j}") for j in range(4)]
            xn2T = [mlpp.tile([128, NQ], BF, tag=f"xn2T{t}", name=f"xn2T{t}") for t in range(CT)]
            hT = [mlpp.tile([128, NQ], BF, tag=f"hT{t}", name=f"hT{t}") for t in range(FT)]
            with (
                pool("mlp1", bufs=1) as m1p,
                pool("mlp1s", bufs=3) as m1s,
            ):
                bfc_bc = m1p.tile([128, DFF], F32, tag="bfcbc", name="bfcbc")
                hh = [m1p.tile([128, DFF], BF, tag=f"hh{j}", name=f"hh{j}") for j in range(4)]
                nc.sync.dma_start(bfc_bc[:], d["d_bfc"][:])

                with (
                    pool("ph4p", bufs=2, space="PSUM") as ph4p,
                    pool("ph4t", bufs=4, space="PSUM") as ph4t,
                ):
                    for qt in range(4):
                        pp = ph4p.tile([128, C], F32, tag="pp", name="pp")
                        for lo, hi in ((0, 512), (512, 768)):
                            for ct in range(CT):
                                nc.tensor.matmul(
                                    pp[:, lo:hi],
                                    yT[ct][:, qt * 128 : (qt + 1) * 128],
                                    wp[ct][:, lo:hi],
                                    start=(ct == 0), stop=(ct == CT - 1),
                                )
                        nc.vector.tensor_add(x2[qt][:], pp[:], x_own[qt][:])
                        mu, rstd = _ln_tile(nc, small, x2[qt][:], eps)
                        xn2 = m1s.tile([128, C], BF, tag="xn2", name="xn2")
                        nc.vector.tensor_scalar(
                            out=xn2[:], in0=x2[qt][:], scalar1=mu, scalar2=rstd[:],
                            op0=ALU.subtract, op1=ALU.mult,
                        )
                        for ct in range(CT):
                            pt = ph4t.tile([128, 128], BF, tag="pt4", name="pt4")
                            nc.tensor.transpose(
                                pt[:], xn2[:, ct * 128 : (ct + 1) * 128], ident[:]
                            )
                            nc.scalar.activation(
                                out=xn2T[ct][:, qt * 128 : (qt + 1) * 128],
                                in_=pt[:], func=AF.Identity,
                                scale=l2g[:, ct : ct + 1], bias=l2b[:, ct : ct + 1],
                            )

                # ======== phase 5: fc + gelu ========
                with pool("ph5p", bufs=2, space="PSUM") as ph5p:
                    for qt in range(4):
                        for nn in range(6):
                            sl = slice(nn * 512, (nn + 1) * 512)
                            ph_ = ph5p.tile([128, 512], F32, tag="ph5", name="ph5")
                            for ct in range(CT):
                                nc.tensor.matmul(
                                    ph_[:],
                                    xn2T[ct][:, qt * 128 : (qt + 1) * 128],
                                    wfc[ct][:, sl],
                                    start=(ct == 0), stop=(ct == CT - 1),
                                )
                            tmp = m1s.tile([128, 512], BF, tag="pregelu", name="pregelu")
                            nc.vector.tensor_add(tmp[:], ph_[:], bfc_bc[:, sl])
                            nc.scalar.activation(
                                out=hh[qt][:, sl], in_=tmp[:], func=AF.Gelu_apprx_tanh,
                            )

                # ======== phase 6: transpose h ========
                with pool("ph6t", bufs=2, space="PSUM") as ph6t:
                    for fc in range(FT):
                        ptb = ph6t.tile([128, 512], BF, tag="pt6", name="pt6")
                        for qt in range(4):
                            nc.tensor.transpose(
                                ptb[:, qt * 128 : (qt + 1) * 128],
                                hh[qt][:, fc * 128 : (fc + 1) * 128], ident[:],
                            )
                        nc.scalar.copy(hT[fc][:], ptb[:])

            # ======== phase 7: fc2 + residual + out ========
            with (
                pool("mlp2", bufs=1) as m2p,
                pool("mlp2s", bufs=3) as m2s,
                pool("ph7p", bufs=2, space="PSUM") as ph7p,
            ):
                wfc2 = [m2p.tile([128, C], BF, tag=f"wfc2{t}", name=f"wfc2{t}") for t in range(FT)]
                for t in range(FT):
                    nc.sync.dma_start(wfc2[t][:], d["d_wfc2"][t * 128 : (t + 1) * 128, :])
                for qt in range(4):
                    po = ph7p.tile([128, C], F32, tag="po", name="po")
                    for lo, hi in ((0, 512), (512, 768)):
                        for kt in range(FT):
                            nc.tensor.matmul(
                                po[:, lo:hi],
                                hT[kt][:, qt * 128 : (qt + 1) * 128],
                                wfc2[kt][:, lo:hi],
                                start=(kt == 0), stop=(kt == FT - 1),
                            )
                    t1 = m2s.tile([128, C], F32, tag="t1", name="t1")
                    nc.vector.tensor_add(t1[:], po[:], bfc2_bc[:])
                    ot = m2s.tile([128, C], F32, tag="ot", name="ot")
                    nc.vector.tensor_add(ot[:], t1[:], x2[qt][:])
                    nc.sync.dma_start(
                        d["d_out"][qt * 128 : (qt + 1) * 128, :], ot[:]
                    )


# ---------------------------------------------------------------------------
# Host-side wrapper
# ---------------------------------------------------------------------------
_PROGRAM = None


def _get_program():
    global _PROGRAM
    if _PROGRAM is None:
        _PROGRAM = build_program()
    return _PROGRAM


def make_in_maps(x, ln1_g, ln1_b, W_attn, b_attn, W_proj, b_proj,
                 ln2_g, ln2_b, W_fc, b_fc, W_fc2, b_fc2):
    x = np.asarray(x, np.float32)
    shared = {
        "wq": pack_dr(W_attn[:, 0:C]),
        "wk": pack_dr(W_attn[:, C : 2 * C]),
        "wv": pack_dr(W_attn[:, 2 * C : 3 * C]),
        "wp": np.asarray(W_proj, BF16),
        "wfc": np.asarray(W_fc, BF16),
        "wfc2": np.asarray(W_fc2, BF16),
        # q/k/v biases ride the 32x weight scale
        "bq2d": np.ascontiguousarray(
            np.asarray(b_attn[0:C], np.float32).reshape(CT, 128).T * WS),
        "bk2d": np.ascontiguousarray(
            np.asarray(b_attn[C : 2 * C], np.float32).reshape(CT, 128).T * WS),
        "bv_bc": np.broadcast_to(
            np.asarray(b_attn[2 * C : 3 * C], np.float32) * WS, (128, C)).copy(),
        "bfc_bc": np.broadcast_to(
            np.asarray(b_fc, np.float32), (128, DFF)).copy(),
        "bfc2_bc": np.broadcast_to(
            np.asarray(b_fc2, np.float32), (128, C)).copy(),
        "ln1g2d": np.ascontiguousarray(
            np.asarray(ln1_g, np.float32).reshape(CT, 128).T),
        "ln1b2d": np.ascontiguousarray(
            np.asarray(ln1_b, np.float32).reshape(CT, 128).T),
        "ln2g2d": np.ascontiguousarray(
            np.asarray(ln2_g, np.float32).reshape(CT, 128).T),
        "ln2b2d": np.ascontiguousarray(
            np.asarray(ln2_b, np.float32).reshape(CT, 128).T),
        "ident": np.eye(128, dtype=BF16),
    }
    bp = np.asarray(b_proj, np.float32)
    in_maps, layouts = [], []
    for core in range(8):
        b, g = core // 4, core % 4
        qtiles, perm = core_layout(g)
        idx = np.concatenate([np.arange(t * 128, (t + 1) * 128) for t in perm])
        own = np.concatenate([np.arange(t * 128, (t + 1) * 128) for t in qtiles])
        m = dict(shared)
        m["x_perm"] = np.ascontiguousarray(x[b][idx])
        m["x_own_b"] = np.ascontiguousarray(x[b][own] + bp)
        m["masks"] = core_masks(qtiles, perm)
        in_maps.append(m)
        layouts.append((b, own))
    return in_maps, layouts


def unshard(results, layouts):
    out = np.empty((B, T, C), np.float32)
    for r, (b, own) in zip(results, layouts):
        out[b][own] = r["out"]
    return out


def kernel(**inputs):
    from concourse.bass_utils import run_bass_kernel_spmd

    nc = _get_program()
    in_maps, layouts = make_in_maps(**inputs)
    res = run_bass_kernel_spmd(nc, in_maps, core_ids=list(range(8)))
    return unshard(res.results, layouts)
